# revision 1
# baseline (speedup 1.0000x reference)
"""Multi-head attention (B=2, S=2048, D=1024, H=16) on 8 TRN2 NeuronCores.

Sharding: tensor-parallel over heads (2 heads/core).  Each core computes
the qkv projection for its heads (full sequence) and attention, then an
AllToAll redistributes attention outputs so each core holds *all* heads
for a 1/8 slice of the (batch*seq) rows and runs the output projection
locally.  No cross-core reduction needed.

Compute dtype: bf16 matmul operands, fp32 PSUM accumulation.  Softmax
denominators come for free from a ones-column appended to V (scores are
small here, so exp without max-subtraction is safe); normalization is a
per-partition scalar multiply fused into the PSUM eviction.

Engines execute their instruction streams in order, so the emission
order below is a hand-software-pipelined schedule: scores/exp of block
i+1 are interleaved with the attention-value matmuls of block i and
with the x-transpose/projection prep of the next batch.
"""

import sys

sys.path.insert(0, "/opt/trn_rl_repo")

import numpy as np
import ml_dtypes

B, S, D = 2, 2048, 1024
H, HD = 16, 64
NCORES = 8
BS = B * S                 # 4096 flattened rows
HL = H // NCORES           # 2 local heads
CH = HL * HD               # 128 local q/k/v channels
ROWS = BS // NCORES        # 512 output rows per core
P = 128
NDC = D // P               # 8 chunks of the contraction dim D
NST = S // P               # 16 seq tiles per batch
NKB = S // P               # 16 key blocks per batch
QCW = 512                  # query-chunk width (one exp instruction per kb)
NQC = S // QCW             # query chunks per batch
HD1 = HD + 1               # value channels + ones column

_CACHE = {}

XPOSE_MODE = "pe"          # "pe" | "dma"


def _interleave(primary, secondary, lead=0):
    """Emit primary tasks in order, spreading secondary tasks between them.
    The first `lead` primary tasks are emitted before any secondary."""
    ns = len(secondary)
    npr = max(len(primary) - lead, 1)
    si = 0
    for i, p in enumerate(primary):
        p()
        tgt = (i + 1 - lead) * ns // npr if i >= lead else 0
        while si < tgt:
            secondary[si]()
            si += 1
    while si < ns:
        secondary[si]()
        si += 1


def _build_program(with_bias: bool, local_a2a: bool = False, xpose: str | None = None,
                   repeats: int = 1, loop_n: int = 0, dve_cast: bool = False,
                   qcw: int = QCW):
    import concourse.bass as bass
    import concourse.mybir as mybir
    import concourse.tile as tile
    from concourse import bacc
    from concourse.masks import make_identity
    from contextlib import ExitStack

    xpose = xpose or XPOSE_MODE
    nqc = S // qcw
    nbb = HL * nqc          # blocks per batch
    dt = mybir.dt
    AF = mybir.ActivationFunctionType
    bf, f32 = dt.bfloat16, dt.float32

    nc = bacc.Bacc()

    x = nc.dram_tensor("x", [BS, D], f32, kind="ExternalInput")
    wq = nc.dram_tensor("wq", [P, NDC, CH], bf, kind="ExternalInput")
    wk = nc.dram_tensor("wk", [P, NDC, CH], bf, kind="ExternalInput")
    wv = nc.dram_tensor("wv", [P, NDC, CH], bf, kind="ExternalInput")
    wo = nc.dram_tensor("wo", [P, NCORES, D], bf, kind="ExternalInput")
    if with_bias:
        bq = nc.dram_tensor("bq", [1, CH], bf, kind="ExternalInput")
        bk = nc.dram_tensor("bk", [1, CH], bf, kind="ExternalInput")
        bv = nc.dram_tensor("bv", [1, CH], bf, kind="ExternalInput")
        ob = nc.dram_tensor("ob", [1, D], bf, kind="ExternalInput")
    y = nc.dram_tensor("y", [ROWS, D], f32, kind="ExternalOutput")

    # weight loads: HWDGE in pe mode; SWDGE in dma mode so the xbar
    # transposes don't interleave with copy-mode HWDGE transfers
    wload = (lambda **kw: nc.sync.dma_start(**kw)) if xpose == "pe" else (
        lambda **kw: nc.gpsimd.dma_start(**kw))

    with tile.TileContext(nc) as tc, ExitStack() as ctx:
        const = ctx.enter_context(tc.tile_pool(name="const", bufs=1))
        ident = const.tile([P, P], bf)
        make_identity(nc, ident[:])

        wq_sb = const.tile([P, NDC, CH], bf)
        wk_sb = const.tile([P, NDC, CH], bf)
        wv_sb = const.tile([P, NDC, CH], bf)
        wo_sb = const.tile([P, NCORES, D], bf)
        if with_bias:
            bq_sb = const.tile([1, CH], bf)
            bk_sb = const.tile([1, CH], bf)
            bv_sb = const.tile([1, CH], bf)
            ob_sb = const.tile([1, D], bf)
            ones_row = const.tile([1, 512], bf)

        big = ctx.enter_context(tc.tile_pool(name="big", bufs=1))
        xT = big.tile([P, NDC, BS], bf)                     # [d%128, d//128, row]
        qT = big.tile([P, BS], bf)                          # q channel-major
        kT = big.tile([P, BS], bf)                          # k channel-major
        v_aug = big.tile([P, B * NST, HL * HD1], bf)        # v row-major + ones
        valsT = big.tile([P, BS], bf)                       # attn out, ch-major

        xin = ctx.enter_context(tc.tile_pool(name="xin", bufs=4))
        expp = ctx.enter_context(tc.tile_pool(name="expp", bufs=(4 if qcw <= 512 else 2)))
        small = ctx.enter_context(tc.tile_pool(name="small", bufs=4))
        outp = ctx.enter_context(tc.tile_pool(name="outp", bufs=2))

        # PSUM budget: 8 banks total; wider score chunks eat banks that
        # otherwise deepen the projection/AV pipelines
        nbank_score = 2 * (qcw // 512)
        nb_big = 1 if nbank_score >= 4 else 2
        nb_av = 1 if nbank_score >= 4 else 2
        pt = ctx.enter_context(tc.tile_pool(name="pt", bufs=2, space="PSUM"))
        pbig = ctx.enter_context(tc.tile_pool(name="pbig", bufs=nb_big, space="PSUM"))
        pscore = ctx.enter_context(tc.tile_pool(name="pscore", bufs=2, space="PSUM"))
        pav = ctx.enter_context(tc.tile_pool(name="pav", bufs=nb_av, space="PSUM"))

        dram = ctx.enter_context(tc.tile_pool(name="dram", bufs=1, space="DRAM"))
        # the AllToAll is split into two half-payload collectives (head 0 /
        # head 1 channel halves) so the first can run under live attention
        ccA_in = dram.tile([NCORES, HD, ROWS], bf)
        ccA_out = dram.tile([NCORES, HD, ROWS], bf)
        ccB_in = dram.tile([NCORES, HD, ROWS], bf)
        ccB_out = dram.tile([NCORES, HD, ROWS], bf)
        if xpose == "dma":
            xbf_dram = dram.tile([BS, D], bf)

        # ones columns for the softmax-denominator trick; value columns are
        # overwritten by the v-projection evictions
        for h in range(HL):
            nc.vector.memset(v_aug[:, :, h * HD1 + HD], 1.0)

        # ---------------- task builders ----------------

        def t_wload(wsb, wdram):
            return lambda: wload(out=wsb[:], in_=wdram[:])

        def t_bias_loads():
            def go():
                wload(out=bq_sb[:], in_=bq[:])
                wload(out=bk_sb[:], in_=bk[:])
                wload(out=bv_sb[:], in_=bv[:])
                wload(out=ob_sb[:], in_=ob[:])
                nc.vector.memset(ones_row[:], 1.0)
            return go

        def t_xpose_pe(st):
            def go():
                x_bf = xin.tile([P, D], bf, tag="xbf", name="x_bf")
                if dve_cast:
                    x_f = xin.tile([P, D], f32, tag="xf", name="x_f", bufs=2)
                    nc.sync.dma_start(out=x_f[:], in_=x[st * P:(st + 1) * P, :])
                    nc.vector.tensor_copy(out=x_bf[:], in_=x_f[:])
                else:
                    nc.gpsimd.dma_start(out=x_bf[:], in_=x[st * P:(st + 1) * P, :])
                for c in range(NDC):
                    ptile = pt.tile([P, P], bf, tag="ptr", name="ptile")
                    nc.tensor.transpose(
                        ptile[:], x_bf[:, c * P:(c + 1) * P], ident[:]
                    )
                    nc.vector.tensor_copy(
                        out=xT[:, c, st * P:(st + 1) * P], in_=ptile[:]
                    )
            return go

        def t_xcast_dma(b, rc):
            def go():
                r0 = b * S + rc * 512
                nc.gpsimd.dma_start(
                    out=xbf_dram[r0:r0 + 512, :], in_=x[r0:r0 + 512, :]
                )
            return go

        def t_xpose_dma(b, rc, c):
            def go():
                r0 = b * S + rc * 512
                nc.sync.dma_start(
                    out=xT[:, c, r0:r0 + 512],
                    in_=xbf_dram[r0:r0 + 512, c * P:(c + 1) * P],
                    transpose=True,
                )
            return go

        def t_vproj(st):
            def go():
                pv = pbig.tile([P, CH], f32, tag="pk", name="pv")
                for c in range(NDC):
                    nc.tensor.matmul(
                        pv[:],
                        lhsT=xT[:, c, st * P:(st + 1) * P],
                        rhs=wv_sb[:, c, :],
                        start=(c == 0),
                        stop=(c == NDC - 1 and not with_bias),
                    )
                if with_bias:
                    nc.tensor.matmul(
                        pv[:], lhsT=ones_row[:, 0:P], rhs=bv_sb[:],
                        start=False, stop=True,
                    )
                for h in range(HL):
                    nc.vector.tensor_copy(
                        out=v_aug[:, st, h * HD1:h * HD1 + HD],
                        in_=pv[:, h * HD:(h + 1) * HD],
                    )
            return go

        def t_kqproj(b, which, qc):
            def go():
                wsb, dst = (wk_sb, kT) if which == "k" else (wq_sb, qT)
                base = b * S + qc * 512
                pq = pbig.tile([P, 512], f32, tag="pk", name="pq")
                for c in range(NDC):
                    nc.tensor.matmul(
                        pq[:],
                        lhsT=wsb[:, c, :],
                        rhs=xT[:, c, base:base + 512],
                        start=(c == 0),
                        stop=(c == NDC - 1 and not with_bias),
                    )
                if with_bias:
                    nc.tensor.matmul(
                        pq[:],
                        lhsT=(bk_sb if which == "k" else bq_sb)[:],
                        rhs=ones_row[:],
                        start=False, stop=True,
                    )
                nc.vector.tensor_copy(out=dst[:, base:base + 512], in_=pq[:])
            return go

        def prep_A_tasks(b):
            """x load/cast/transpose + v projection; one task pair per seq
            tile (returned flat, in order)."""
            tasks = []
            if xpose == "pe":
                for t in range(NST):
                    tasks.append(t_xpose_pe(b * NST + t))
                    tasks.append(t_vproj(b * NST + t))
            else:
                for rc in range(4):
                    tasks.append(t_xcast_dma(b, rc))
                    for c in range(NDC):
                        tasks.append(t_xpose_dma(b, rc, c))
                    for tt in range(4):
                        tasks.append(t_vproj(b * NST + rc * 4 + tt))
            return tasks

        # attention blocks: per (b, h, qc) -> score tasks (one per kb) and
        # AV tasks (one per qt)
        def score_tasks(b, h, qc, et):
            hp = h * HD
            qbase = b * S + qc * qcw
            tasks = []

            def mk(kb):
                def go():
                    kbase = b * S + kb * P
                    ps = pscore.tile([P, qcw], f32, tag="ps", name="ps")
                    for qh in range(qcw // 512):
                        nc.tensor.matmul(
                            ps[:, qh * 512:(qh + 1) * 512],
                            lhsT=kT[hp:hp + HD, kbase:kbase + P],
                            rhs=qT[hp:hp + HD,
                                   qbase + qh * 512:qbase + (qh + 1) * 512],
                            start=True,
                            stop=True,
                        )
                    nc.scalar.activation(et[:, kb, :], ps[:], AF.Exp, scale=0.125)
                return go

            for kb in range(NKB):
                tasks.append(mk(kb))
            return tasks

        def av_tasks(b, h, qc, et):
            hp = h * HD
            qbase = b * S + qc * qcw
            tasks = []

            def mk(qt):
                def go():
                    pa = pav.tile([P, HD1], f32, tag="pa", name="pa")
                    for kb in range(NKB):
                        nc.tensor.matmul(
                            pa[:],
                            lhsT=et[:, kb, qt * P:(qt + 1) * P],
                            rhs=v_aug[:, b * NKB + kb, h * HD1:(h + 1) * HD1],
                            start=(kb == 0),
                            stop=(kb == NKB - 1),
                        )
                    rc_ = small.tile([P, 1], f32, tag="rc", name="rc")
                    nc.vector.reciprocal(rc_[:], pa[:, HD:HD1])
                    vn = small.tile([P, HD], bf, tag="vn", name="vn")
                    nc.vector.tensor_scalar_mul(vn[:], pa[:, 0:HD], rc_[:])
                    ptv = pt.tile([P, P], bf, tag="ptr", name="ptv")
                    nc.tensor.transpose(ptv[hp:hp + HD, :], vn[:], ident[:])
                    col = qbase + qt * P
                    nc.vector.tensor_copy(
                        out=valsT[hp:hp + HD, col:col + P],
                        in_=ptv[hp:hp + HD, :],
                    )
                return go

            for qt in range(qcw // P):
                tasks.append(mk(qt))
            return tasks

        def t_ccdma(half, j):
            ccin = ccA_in if half == 0 else ccB_in
            hp = half * HD
            return lambda: nc.sync.dma_start(
                out=ccin[j], in_=valsT[hp:hp + HD, j * ROWS:(j + 1) * ROWS]
            )

        def t_a2a(half):
            ccin, ccout = (ccA_in, ccA_out) if half == 0 else (ccB_in, ccB_out)

            def go():
                if local_a2a:
                    nc.sync.dma_start(out=ccout[:], in_=ccin[:])
                else:
                    nc.gpsimd.collective_compute(
                        "AllToAll",
                        mybir.AluOpType.bypass,
                        replica_groups=[list(range(NCORES))],
                        ins=[ccin[:]],
                        outs=[ccout[:]],
                    )
            return go

        # ---------------- emission (software pipeline) ----------------
        def emit_body(load_weights):
            if load_weights:
                t_wload(wv_sb, wv)()
                t_wload(wk_sb, wk)()
                t_wload(wq_sb, wq)()
                if with_bias:
                    t_bias_loads()()

            A0 = prep_A_tasks(0)        # per seq tile: [xpose, vproj] pairs
            A1 = prep_A_tasks(1)
            # front: enough of batch 0 to start scoring, k/q chunks woven in
            if xpose == "pe":
                for task in A0[0:8]:    # seq tiles 0..3
                    task()
                t_kqproj(0, "k", 0)()
                for task in A0[8:16]:   # seq tiles 4..7
                    task()
            else:
                for task in A0:
                    task()
                t_kqproj(0, "k", 0)()
            t_kqproj(0, "q", 0)()
            if qcw > 512:
                t_kqproj(0, "q", 1)()

            # h-major block order per batch: the head-0 half of valsT is
            # complete after the last (b1,h0,*) block, letting the first
            # half-AllToAll run under the remaining head-1 attention.
            block_ids = [(b, h, qc) for b in range(B) for h in range(HL)
                         for qc in range(nqc)]
            nblk = len(block_ids)
            warm1 = [t_kqproj(1, "k", 0), t_kqproj(1, "q", 0)]
            if qcw > 512:
                warm1.append(t_kqproj(1, "q", 1))

            vfull = big.tile([P, NCORES, ROWS], bf, name="vfull")

            def t_vfull(half):
                ccout = ccA_out if half == 0 else ccB_out
                hp = half * HD
                return lambda: nc.sync.dma_start(
                    out=vfull[hp:hp + HD, :, :],
                    in_=ccout.rearrange("i p r -> p i r"),
                )

            # extra tasks joining the mix at a given global block index
            # (cc DMAs depend on AV tasks which lag their block by one)
            from collections import defaultdict
            extras = defaultdict(list)
            tail_tasks = []

            def sched(idx, task):
                if idx < nblk:
                    extras[idx].append(task)
                else:
                    tail_tasks.append(task)

            lastA = 0
            for b in range(B):
                for q in range(4):            # 512-row slot quarters
                    j = b * 4 + q
                    qc_of = q * 512 // qcw
                    blkA = b * nbb + qc_of
                    blkB = b * nbb + nqc + qc_of
                    sched(blkA + 2, t_ccdma(0, j))
                    sched(blkB + 2, t_ccdma(1, j))
                    lastA = max(lastA, blkA + 2)
            sched(lastA, t_a2a(0))
            sched(lastA, t_vfull(0))
            tail_tasks += [t_a2a(1), t_vfull(1)]
            if load_weights:
                sched(nbb, t_wload(wo_sb, wo))
            # q-projection chunk c is emitted one block before the first
            # (b, h0, qc) block that reads it (chunk 0 — and 1 for wide
            # chunks — comes from the front / warm1 instead)
            for b in range(B):
                for c in range(1, 4):
                    first_qc = c * 512 // qcw
                    if first_qc == 0:
                        continue
                    extras[b * nbb + first_qc - 1].insert(
                        0, t_kqproj(b, "q", c))

            prev_av = []
            for i, (b, h, qc) in enumerate(block_ids):
                et = expp.tile([P, NKB, qcw], bf, tag="exp", name="et")
                s = score_tasks(b, h, qc, et)
                if h == 0 and qc == 0:
                    kp = [t_kqproj(b, "k", c) for c in (1, 2, 3)]
                    if b == 0 and xpose == "pe":
                        # explicit weave: remaining A tiles + k chunks after
                        # the A tiles they contract over
                        # (scores kb 4c..4c+3 need k chunk c <- A tiles 4c..4c+3)
                        primary = (s[0:2] + A0[16:20] + s[2:4] + kp[0:1]
                                   + A0[20:24] + s[4:6] + A0[24:28] + s[6:8]
                                   + kp[1:2] + A0[28:32] + s[8:12] + kp[2:3]
                                   + s[12:16])
                    else:
                        primary = (s[0:4] + kp[0:1] + s[4:8] + kp[1:2]
                                   + s[8:12] + kp[2:3] + s[12:16])
                else:
                    primary = s
                mix = extras.get(i, [])[:]
                mix += prev_av
                if b == 0 and 1 <= i <= nbb - 2:
                    lo = (i - 1) * len(A1) // (nbb - 2)
                    hi = i * len(A1) // (nbb - 2)
                    mix += A1[lo:hi]
                    if i == nbb - 2:
                        mix += warm1
                _interleave(primary, mix, lead=2)
                prev_av = av_tasks(b, h, qc, et)
            for task in prev_av:
                task()
            for task in tail_tasks:
                task()

            # ---- output projection ----
            for rt in range(ROWS // P):
                for dh in range(D // 512):
                    po = pscore.tile([P, 512], f32, tag="ps", name="po")
                    for c in range(NCORES):
                        nc.tensor.matmul(
                            po[:],
                            lhsT=vfull[:, c, rt * P:(rt + 1) * P],
                            rhs=wo_sb[:, c, dh * 512:(dh + 1) * 512],
                            start=(c == 0),
                            stop=(c == NCORES - 1 and not with_bias),
                        )
                    if with_bias:
                        nc.tensor.matmul(
                            po[:], lhsT=ones_row[:, 0:P],
                            rhs=ob_sb[:, dh * 512:(dh + 1) * 512],
                            start=False, stop=True,
                        )
                    osb = outp.tile([P, 512], f32, tag="osb", name="osb")
                    nc.vector.tensor_copy(out=osb[:], in_=po[:])
                    nc.sync.dma_start(
                        out=y[rt * P:(rt + 1) * P, dh * 512:(dh + 1) * 512],
                        in_=osb[:],
                    )

        if loop_n > 1:
            t_wload(wv_sb, wv)()
            t_wload(wk_sb, wk)()
            t_wload(wq_sb, wq)()
            t_wload(wo_sb, wo)()
            if with_bias:
                t_bias_loads()()
            with tc.For_i(0, loop_n, 1):
                emit_body(load_weights=False)
        else:
            for rep in range(repeats):
                emit_body(load_weights=(rep == 0))

    nc.compile()
    return nc


def get_program(with_bias: bool, local_a2a: bool = False, xpose: str | None = None,
                repeats: int = 1, loop_n: int = 0, dve_cast: bool = False,
                qcw: int = QCW):
    key = (with_bias, local_a2a, xpose or XPOSE_MODE, repeats, loop_n, dve_cast, qcw)
    if key not in _CACHE:
        _CACHE[key] = _build_program(with_bias, local_a2a, xpose, repeats, loop_n,
                                     dve_cast, qcw)
    return _CACHE[key]


def make_in_maps(x, qkv_w, qkv_b, o_w, o_b):
    """Host-side sharding: slice per-head weight rows, transpose to the
    layouts the kernel consumes, cast weights to bf16."""
    bfnp = ml_dtypes.bfloat16
    x2 = np.ascontiguousarray(np.asarray(x, np.float32).reshape(BS, D))

    qkv_w = np.asarray(qkv_w, np.float32)
    o_w = np.asarray(o_w, np.float32)
    qkv_b = np.asarray(qkv_b, np.float32)
    o_b = np.asarray(o_b, np.float32)

    with_bias = bool(np.any(qkv_b) or np.any(o_b))

    woT = np.ascontiguousarray(
        o_w.T.reshape(NCORES, P, D).transpose(1, 0, 2).astype(bfnp)
    )
    ob_host = np.ascontiguousarray(o_b.reshape(1, D).astype(bfnp))

    in_maps = []
    for m in range(NCORES):
        heads = [m * HL + h for h in range(HL)]
        q_rows = np.concatenate([qkv_w[h * 3 * HD:h * 3 * HD + HD] for h in heads])
        k_rows = np.concatenate(
            [qkv_w[h * 3 * HD + HD:h * 3 * HD + 2 * HD] for h in heads]
        )
        v_rows = np.concatenate(
            [qkv_w[h * 3 * HD + 2 * HD:h * 3 * HD + 3 * HD] for h in heads]
        )

        def wt(rows):
            # [CH, D] -> [D, CH] -> [p, chunk, CH]
            return np.ascontiguousarray(
                rows.T.reshape(NDC, P, CH).transpose(1, 0, 2).astype(bfnp)
            )

        im = {
            "x": x2,
            "wq": wt(q_rows),
            "wk": wt(k_rows),
            "wv": wt(v_rows),
            "wo": woT,
        }
        if with_bias:
            bqv = np.concatenate(
                [qkv_b[h * 3 * HD:h * 3 * HD + HD] for h in heads]
            )
            bkv = np.concatenate(
                [qkv_b[h * 3 * HD + HD:h * 3 * HD + 2 * HD] for h in heads]
            )
            bvv = np.concatenate(
                [qkv_b[h * 3 * HD + 2 * HD:h * 3 * HD + 3 * HD] for h in heads]
            )
            im["bq"] = np.ascontiguousarray(bqv.reshape(1, CH).astype(bfnp))
            im["bk"] = np.ascontiguousarray(bkv.reshape(1, CH).astype(bfnp))
            im["bv"] = np.ascontiguousarray(bvv.reshape(1, CH).astype(bfnp))
            im["ob"] = ob_host
        in_maps.append(im)
    return in_maps, with_bias


def kernel(x, qkv_w, qkv_b, o_w, o_b):
    from concourse.bass_utils import run_bass_kernel_spmd

    in_maps, with_bias = make_in_maps(x, qkv_w, qkv_b, o_w, o_b)
    nc = get_program(with_bias)
    res = run_bass_kernel_spmd(nc, in_maps, list(range(NCORES)))
    out = np.concatenate([res.results[m]["y"] for m in range(NCORES)], axis=0)
    return np.ascontiguousarray(out.reshape(B, S, D))



# revision 41
# speedup vs baseline: 5.6405x; 5.6405x over previous
"""Multi-head attention (B=2, S=2048, D=1024, H=16) on 8 TRN2 NeuronCores.

Sharding: tensor-parallel over heads (2 heads/core).  Each core computes
the qkv projection for its heads (full sequence) and attention, then an
AllToAll redistributes attention outputs so each core holds *all* heads
for a 1/8 slice of the (batch*seq) rows and runs the output projection
locally.  No cross-core reduction needed.

Compute dtype: bf16 matmul operands, fp32 PSUM accumulation.  Softmax
denominators come for free from a ones-column appended to V (scores are
small here, so exp without max-subtraction is safe); normalization is a
per-partition scalar multiply fused into the PSUM eviction.

Engines execute their instruction streams in order, so the emission
order below is a hand-software-pipelined schedule: scores/exp of block
i+1 are interleaved with the attention-value matmuls of block i and
with the x-transpose/projection prep of the next batch.
"""

import sys

sys.path.insert(0, "/opt/trn_rl_repo")

import numpy as np
import ml_dtypes

B, S, D = 2, 2048, 1024
H, HD = 16, 64
NCORES = 8
BS = B * S                 # 4096 flattened rows
HL = H // NCORES           # 2 local heads
CH = HL * HD               # 128 local q/k/v channels
ROWS = BS // NCORES        # 512 output rows per core
P = 128
NDC = D // P               # 8 chunks of the contraction dim D
NST = S // P               # 16 seq tiles per batch
NKB = S // P               # 16 key blocks per batch
QCW = 1024                 # query-chunk width (one exp instruction per kb)
NQC = S // QCW             # query chunks per batch
HD1 = HD + 1               # value channels + ones column

_CACHE = {}

XPOSE_MODE = "pe"          # "pe" | "dma"


def _interleave(primary, secondary, lead=0):
    """Emit primary tasks in order, spreading secondary tasks between them.
    The first `lead` primary tasks are emitted before any secondary."""
    ns = len(secondary)
    npr = max(len(primary) - lead, 1)
    si = 0
    for i, p in enumerate(primary):
        p()
        tgt = (i + 1 - lead) * ns // npr if i >= lead else 0
        while si < tgt:
            secondary[si]()
            si += 1
    while si < ns:
        secondary[si]()
        si += 1


def _build_program(with_bias: bool, local_a2a: bool = False, xpose: str | None = None,
                   repeats: int = 1, loop_n: int = 0, dve_cast: bool = False,
                   qcw: int = QCW):
    import concourse.bass as bass
    import concourse.mybir as mybir
    import concourse.tile as tile
    from concourse import bacc
    from concourse.masks import make_identity
    from contextlib import ExitStack

    xpose = xpose or XPOSE_MODE
    nqc = S // qcw
    nbb = HL * nqc          # blocks per batch
    dt = mybir.dt
    AF = mybir.ActivationFunctionType
    bf, f32 = dt.bfloat16, dt.float32

    nc = bacc.Bacc()

    x = nc.dram_tensor("x", [BS, D], f32, kind="ExternalInput")
    wq = nc.dram_tensor("wq", [P, NDC, CH], bf, kind="ExternalInput")
    wk = nc.dram_tensor("wk", [P, NDC, CH], bf, kind="ExternalInput")
    wv = nc.dram_tensor("wv", [P, NDC, CH], bf, kind="ExternalInput")
    wo = nc.dram_tensor("wo", [P, NCORES, D], bf, kind="ExternalInput")
    if with_bias:
        bq = nc.dram_tensor("bq", [1, CH], bf, kind="ExternalInput")
        bk = nc.dram_tensor("bk", [1, CH], bf, kind="ExternalInput")
        bv = nc.dram_tensor("bv", [1, CH], bf, kind="ExternalInput")
        ob = nc.dram_tensor("ob", [1, D], bf, kind="ExternalInput")
    y = nc.dram_tensor("y", [ROWS, D], f32, kind="ExternalOutput")

    # weight loads: HWDGE in pe mode; SWDGE in dma mode so the xbar
    # transposes don't interleave with copy-mode HWDGE transfers
    wload = (lambda **kw: nc.sync.dma_start(**kw)) if xpose == "pe" else (
        lambda **kw: nc.gpsimd.dma_start(**kw))

    with tile.TileContext(nc) as tc, ExitStack() as ctx:
        const = ctx.enter_context(tc.tile_pool(name="const", bufs=1))
        ident = const.tile([P, P], bf)
        make_identity(nc, ident[:])

        wq_sb = const.tile([P, NDC, CH], bf)
        wk_sb = const.tile([P, NDC, CH], bf)
        wv_sb = const.tile([P, NDC, CH], bf)
        wo_sb = const.tile([P, NCORES, D], bf)
        if with_bias:
            bq_sb = const.tile([1, CH], bf)
            bk_sb = const.tile([1, CH], bf)
            bv_sb = const.tile([1, CH], bf)
            ob_sb = const.tile([1, D], bf)
            ones_row = const.tile([1, 512], bf)

        big = ctx.enter_context(tc.tile_pool(name="big", bufs=1))
        xT = big.tile([P, NDC, BS], bf)                     # [d%128, d//128, row]
        qT = big.tile([P, BS], bf)                          # q channel-major
        kT = big.tile([P, BS], bf)                          # k channel-major
        v_aug = big.tile([P, B * NST, HL * HD1], bf)        # v row-major + ones
        valsT = big.tile([P, BS], bf)                       # attn out, ch-major

        xin = ctx.enter_context(tc.tile_pool(name="xin", bufs=4))
        expp = ctx.enter_context(tc.tile_pool(name="expp", bufs=(4 if qcw <= 512 else 2)))
        small = ctx.enter_context(tc.tile_pool(name="small", bufs=4))
        outp = ctx.enter_context(tc.tile_pool(name="outp", bufs=4))

        # PSUM budget: 8 banks total (bank-granular per buffer).
        #   qcw=512:  pt 2 + score 3 + proj 1 + av 2 = 8
        #   qcw=1024: pt 1 + score 2x2 + proj 1 + av 2 = 8
        nb_pt = 2 if qcw == 512 else 1
        nb_sc = 3 if qcw == 512 else 2
        pt = ctx.enter_context(tc.tile_pool(name="pt", bufs=nb_pt, space="PSUM"))
        pscore = ctx.enter_context(tc.tile_pool(name="pscore", bufs=nb_sc, space="PSUM"))
        pbig = ctx.enter_context(tc.tile_pool(name="pbig", bufs=1, space="PSUM"))
        pav = ctx.enter_context(tc.tile_pool(name="pav", bufs=2, space="PSUM"))

        dram = ctx.enter_context(tc.tile_pool(name="dram", bufs=1, space="DRAM"))
        # the AllToAll is split into two half-payload collectives (head 0 /
        # head 1 channel halves) so the first can run under live attention
        ccA_in = dram.tile([NCORES, HD, ROWS], bf)
        ccA_out = dram.tile([NCORES, HD, ROWS], bf)
        ccB_in = dram.tile([NCORES, HD, ROWS], bf)
        ccB_out = dram.tile([NCORES, HD, ROWS], bf)
        if xpose in ("dma", "hybrid"):
            xbf_dram = dram.tile([BS, D], bf)

        # ones columns for the softmax-denominator trick; value columns are
        # overwritten by the v-projection evictions
        for h in range(HL):
            nc.vector.memset(v_aug[:, :, h * HD1 + HD], 1.0)

        # ---------------- task builders ----------------

        def t_wload(wsb, wdram):
            return lambda: wload(out=wsb[:], in_=wdram[:])

        def t_bias_loads():
            def go():
                wload(out=bq_sb[:], in_=bq[:])
                wload(out=bk_sb[:], in_=bk[:])
                wload(out=bv_sb[:], in_=bv[:])
                wload(out=ob_sb[:], in_=ob[:])
                nc.vector.memset(ones_row[:], 1.0)
            return go

        xbufs = {}

        def t_xload(st):
            def go():
                x_bf = xin.tile([P, D], bf, tag="xbf", name="x_bf")
                xbufs[st] = x_bf
                nc.gpsimd.dma_start(out=x_bf[:], in_=x[st * P:(st + 1) * P, :])
            return go

        def t_xpose_pe(st):
            def go():
                x_bf = xbufs.pop(st)
                ptile = pt.tile([P, D], bf, tag="ptr", name="ptile")
                for c in range(NDC):
                    nc.tensor.transpose(
                        ptile[:, c * P:(c + 1) * P],
                        x_bf[:, c * P:(c + 1) * P], ident[:]
                    )
                # one wide PSUM->SBUF eviction per seq tile; dst free dims
                # (chunk, row-in-tile) match ptile's column order
                nc.vector.tensor_copy(
                    out=xT[:, :, st * P:(st + 1) * P], in_=ptile[:]
                )
            return go

        def t_xcast_dma(b, rc):
            def go():
                r0 = b * S + rc * 512
                nc.gpsimd.dma_start(
                    out=xbf_dram[r0:r0 + 512, :], in_=x[r0:r0 + 512, :]
                )
            return go

        def t_xpose_dma(b, rc, c):
            def go():
                r0 = b * S + rc * 512
                nc.sync.dma_start(
                    out=xT[:, c, r0:r0 + 512],
                    in_=xbf_dram[r0:r0 + 512, c * P:(c + 1) * P],
                    transpose=True,
                )
            return go

        def t_vproj4_parts(g):
            """v projection for seq tiles 4g..4g+3: four subtasks sharing
            one PSUM bank (disjoint 128-col accumulation groups); one
            strided eviction on the last.  NOTE: no other 'pk'-ring
            allocation may be emitted between the parts (single-buffer
            ring would head-of-line block the PE stream)."""
            state = {}

            def part(k):
                def go():
                    if k == 0:
                        state["pv"] = pbig.tile([P, 512], f32, tag="pk",
                                                name="pv4")
                    pv = state["pv"]
                    st = g * 4 + k
                    cs = pv[:, k * P:(k + 1) * P]
                    for c in range(NDC):
                        nc.tensor.matmul(
                            cs,
                            lhsT=xT[:, c, st * P:(st + 1) * P],
                            rhs=wv_sb[:, c, :],
                            start=(c == 0),
                            stop=(c == NDC - 1 and not with_bias),
                        )
                    if with_bias:
                        nc.tensor.matmul(
                            cs, lhsT=ones_row[:, 0:P], rhs=bv_sb[:],
                            start=False, stop=True,
                        )
                    if k == 3:
                        # out free dims (st, h, ch) / in (k, h, ch)
                        nc.vector.tensor_copy(
                            out=v_aug[:, g * 4:(g + 1) * 4, 0:HL * HD1]
                                .rearrange("p s (h c) -> p s h c",
                                           h=HL)[:, :, :, 0:HD],
                            in_=pv[:].rearrange("p (k h c) -> p k h c",
                                                k=4, h=HL),
                        )
                return go

            return [part(k) for k in range(4)]

        def t_kqproj(b, which, qc):
            def go():
                wsb, dst = (wk_sb, kT) if which == "k" else (wq_sb, qT)
                base = b * S + qc * 512
                pq = pbig.tile([P, 512], f32, tag="pk", name="pq")
                for c in range(NDC):
                    nc.tensor.matmul(
                        pq[:],
                        lhsT=wsb[:, c, :],
                        rhs=xT[:, c, base:base + 512],
                        start=(c == 0),
                        stop=(c == NDC - 1 and not with_bias),
                    )
                if with_bias:
                    nc.tensor.matmul(
                        pq[:],
                        lhsT=(bk_sb if which == "k" else bq_sb)[:],
                        rhs=ones_row[:],
                        start=False, stop=True,
                    )
                nc.vector.tensor_copy(out=dst[:, base:base + 512], in_=pq[:])
            return go

        def prep_A_tasks(b):
            """x load/cast/transpose + v projection, as (prefix, groups):
            groups[g] ends with the (split) v projection of seq tiles
            4g..4g+3 and all their transposes."""
            mode = xpose if xpose != "hybrid" else ("pe" if b == 0 else "dma")
            prefix, groups = [], []
            if mode == "pe":
                sts = [b * NST + t for t in range(NST)]
                prefix = [t_xload(sts[0]), t_xload(sts[1])]
                for g in range(4):
                    gt = []
                    for k in range(4):
                        i = g * 4 + k
                        if i + 2 < NST:
                            gt.append(t_xload(sts[i + 2]))
                        gt.append(t_xpose_pe(sts[i]))
                    gt += t_vproj4_parts(b * 4 + g)
                    groups.append(gt)
            else:
                for rc in range(4):
                    gt = [t_xcast_dma(b, rc)]
                    for c in range(NDC):
                        gt.append(t_xpose_dma(b, rc, c))
                    gt += t_vproj4_parts(b * 4 + rc)
                    groups.append(gt)
            return prefix, groups

        # attention blocks: per (b, h, qc) -> score tasks (one per kb) and
        # AV tasks (one per qt)
        def score_tasks(b, h, qc, et):
            hp = h * HD
            qbase = b * S + qc * qcw
            tasks = []

            def mk(kb):
                def go():
                    kbase = b * S + kb * P
                    ps = pscore.tile([P, qcw], f32, tag="ps", name="ps")
                    for qh in range(qcw // 512):
                        nc.tensor.matmul(
                            ps[:, qh * 512:(qh + 1) * 512],
                            lhsT=kT[hp:hp + HD, kbase:kbase + P],
                            rhs=qT[hp:hp + HD,
                                   qbase + qh * 512:qbase + (qh + 1) * 512],
                            start=True,
                            stop=True,
                        )
                    nc.scalar.activation(et[:, kb, :], ps[:], AF.Exp, scale=0.125)
                return go

            for kb in range(NKB):
                tasks.append(mk(kb))
            return tasks

        def av_tasks(b, h, qc, et):
            hp = h * HD
            qbase = b * S + qc * qcw
            tasks = []

            def mk(qt):
                def go():
                    pa = pav.tile([P, HD1], f32, tag="pa", name="pa")
                    for kb in range(NKB):
                        nc.tensor.matmul(
                            pa[:],
                            lhsT=et[:, kb, qt * P:(qt + 1) * P],
                            rhs=v_aug[:, b * NKB + kb, h * HD1:(h + 1) * HD1],
                            start=(kb == 0),
                            stop=(kb == NKB - 1),
                        )
                    rc_ = small.tile([P, 1], f32, tag="rc", name="rc")
                    nc.vector.reciprocal(rc_[:], pa[:, HD:HD1])
                    vn = small.tile([P, HD], bf, tag="vn", name="vn")
                    nc.vector.tensor_scalar_mul(vn[:], pa[:, 0:HD], rc_[:])
                    ptv = pt.tile([P, D], bf, tag="ptr", name="ptv")
                    nc.tensor.transpose(ptv[hp:hp + HD, 0:P], vn[:], ident[:])
                    col = qbase + qt * P
                    nc.vector.tensor_copy(
                        out=valsT[hp:hp + HD, col:col + P],
                        in_=ptv[hp:hp + HD, 0:P],
                    )
                return go

            for qt in range(qcw // P):
                tasks.append(mk(qt))
            return tasks

        def t_ccdma(half, j):
            ccin = ccA_in if half == 0 else ccB_in
            hp = half * HD
            return lambda: nc.sync.dma_start(
                out=ccin[j], in_=valsT[hp:hp + HD, j * ROWS:(j + 1) * ROWS]
            )

        def t_a2a(half):
            ccin, ccout = (ccA_in, ccA_out) if half == 0 else (ccB_in, ccB_out)

            def go():
                if local_a2a:
                    nc.sync.dma_start(out=ccout[:], in_=ccin[:])
                else:
                    nc.gpsimd.collective_compute(
                        "AllToAll",
                        mybir.AluOpType.bypass,
                        replica_groups=[list(range(NCORES))],
                        ins=[ccin[:]],
                        outs=[ccout[:]],
                    )
            return go

        # ---------------- emission (software pipeline) ----------------
        def emit_body(load_weights):
            A0 = prep_A_tasks(0)        # per seq tile: [xpose, vproj] pairs
            # first x loads go out before the weight loads: they gate the
            # whole front, and run on the SWDGE queue anyway
            A0[0]()
            A0[1]()
            A0 = A0[2:]
            if load_weights:
                t_wload(wv_sb, wv)()
                t_wload(wk_sb, wk)()
                t_wload(wq_sb, wq)()
                if with_bias:
                    t_bias_loads()()
            if xpose == "hybrid":
                # split b1 prep: DMA casts+transposes go early (pure DMA,
                # don't enter the PE stream); vprojs go late so the PE
                # never head-of-line blocks on a pending DMA transpose
                A1_dma, A1_vp = [], []
                for rc in range(4):
                    A1_dma.append(t_xcast_dma(1, rc))
                    for c in range(NDC):
                        A1_dma.append(t_xpose_dma(1, rc, c))
                for g in range(4):
                    A1_vp.append(t_vproj4(4 + g))
                A1 = []
            else:
                A1 = prep_A_tasks(1)
            # front: enough of batch 0 to start scoring, k/q chunks woven in
            if xpose in ("pe", "hybrid"):
                for task in A0[0:11]:   # seq tiles 0..3 (+ loads ahead)
                    task()
                t_kqproj(0, "k", 0)()
                for task in A0[11:20]:  # seq tiles 4..7
                    task()
            else:
                for task in A0:
                    task()
                t_kqproj(0, "k", 0)()
            t_kqproj(0, "q", 0)()
            if qcw > 512:
                t_kqproj(0, "q", 1)()

            # h-major block order per batch: the head-0 half of valsT is
            # complete after the last (b1,h0,*) block, letting the first
            # half-AllToAll run under the remaining head-1 attention.
            block_ids = [(b, h, qc) for b in range(B) for h in range(HL)
                         for qc in range(nqc)]
            nblk = len(block_ids)
            warm1 = [t_kqproj(1, "k", 0), t_kqproj(1, "q", 0)]
            if qcw > 512:
                warm1.append(t_kqproj(1, "q", 1))

            vfull = big.tile([P, NCORES, ROWS], bf, name="vfull")

            def t_vfull(half):
                ccout = ccA_out if half == 0 else ccB_out
                hp = half * HD
                return lambda: nc.sync.dma_start(
                    out=vfull[hp:hp + HD, :, :],
                    in_=ccout.rearrange("i p r -> p i r"),
                )

            def t_vfull_rows(half, rt):
                ccout = ccA_out if half == 0 else ccB_out
                hp = half * HD
                return lambda: nc.sync.dma_start(
                    out=vfull[hp:hp + HD, :, rt * P:(rt + 1) * P],
                    in_=ccout[:, :, rt * P:(rt + 1) * P].rearrange(
                        "i p r -> p i r"),
                )

            def oproj_rt(rt):
                def go():
                    for dh in range(D // 512):
                        if qcw == 512:
                            # spread the 8 tiles over both [128,512] f32
                            # rings so the PE never waits on an eviction
                            pool, tg = ((pbig, "pk") if (rt * 2 + dh) % 4 == 3
                                        else (pscore, "ps"))
                            po = pool.tile([P, 512], f32, tag=tg, name="po")
                            pslice = po[:]
                        else:
                            if dh == 0:
                                po = pscore.tile([P, qcw], f32, tag="ps",
                                                 name="po")
                            pslice = po[:, dh * 512:(dh + 1) * 512]
                        for c in range(NCORES):
                            nc.tensor.matmul(
                                pslice,
                                lhsT=vfull[:, c, rt * P:(rt + 1) * P],
                                rhs=wo_sb[:, c, dh * 512:(dh + 1) * 512],
                                start=(c == 0),
                                stop=(c == NCORES - 1 and not with_bias),
                            )
                        if with_bias:
                            nc.tensor.matmul(
                                pslice, lhsT=ones_row[:, 0:P],
                                rhs=ob_sb[:, dh * 512:(dh + 1) * 512],
                                start=False, stop=True,
                            )
                        osb = outp.tile([P, 512], f32, tag="osb", name="osb")
                        nc.vector.tensor_copy(out=osb[:], in_=pslice)
                        # y writes go out on the ACT HWDGE queue so they
                        # never head-of-line block vfull loads on SP
                        nc.scalar.dma_start(
                            out=y[rt * P:(rt + 1) * P,
                                  dh * 512:(dh + 1) * 512],
                            in_=osb[:],
                        )
                return go

            # extra tasks joining the mix at a given global block index
            # (cc DMAs depend on AV tasks which lag their block by one)
            from collections import defaultdict
            extras = defaultdict(list)
            tail_tasks = []

            def sched(idx, task):
                if idx < nblk:
                    extras[idx].append(task)
                else:
                    tail_tasks.append(task)

            lastA = 0
            late_cc = []
            for b in range(B):
                for q in range(4):            # 512-row slot quarters
                    j = b * 4 + q
                    qc_of = q * 512 // qcw
                    blkA = b * nbb + qc_of
                    blkB = b * nbb + nqc + qc_of
                    sched(blkA + 2, t_ccdma(0, j))
                    if blkB + 2 < nblk:
                        sched(blkB + 2, t_ccdma(1, j))
                    else:
                        late_cc.append((blkB + 2, t_ccdma(1, j)))
                    lastA = max(lastA, blkA + 2)
            sched(lastA, t_a2a(0))
            sched(lastA, t_vfull(0))
            if load_weights:
                sched(nbb, t_wload(wo_sb, wo))
            # q-projection chunk c is emitted one block before the first
            # (b, h0, qc) block that reads it (chunk 0 — and 1 for wide
            # chunks — comes from the front / warm1 instead)
            for b in range(B):
                for c in range(1, 4):
                    first_qc = c * 512 // qcw
                    if first_qc == 0:
                        continue
                    if (b == 0 and first_qc == 1 and qcw > 512
                            and xpose in ("pe", "hybrid")):
                        # block 0's mix interleaves from the start — these
                        # chunks contract seq tiles transposed late in block
                        # 0's primary, so they're woven there instead
                        continue
                    extras[b * nbb + first_qc - 1].insert(
                        0, t_kqproj(b, "q", c))

            prev_av = []
            for i, (b, h, qc) in enumerate(block_ids):
                et = expp.tile([P, NKB, qcw], bf, tag="exp", name="et")
                s = score_tasks(b, h, qc, et)
                if h == 0 and qc == 0:
                    kp = [t_kqproj(b, "k", c) for c in (1, 2, 3)]
                    if b == 0 and xpose in ("pe", "hybrid"):
                        # explicit weave: remaining A tiles + k chunks after
                        # the A tiles they contract over
                        # (scores kb 4c..4c+3 need k chunk c <- A tiles 4c..4c+3)
                        qp = ([t_kqproj(0, "q", 2), t_kqproj(0, "q", 3)]
                              if qcw > 512 else [])
                        primary = (s[0:2] + A0[20:24] + s[2:4] + kp[0:1]
                                   + A0[24:29] + s[4:6] + A0[29:33] + s[6:8]
                                   + kp[1:2] + A0[33:36] + qp + s[8:12]
                                   + kp[2:3] + s[12:16])
                    else:
                        primary = (s[0:4] + kp[0:1] + s[4:8] + kp[1:2]
                                   + s[8:12] + kp[2:3] + s[12:16])
                else:
                    primary = s
                mix = extras.get(i, [])[:]
                mix += prev_av
                if xpose == "hybrid" and b == 0:
                    if 1 <= i <= 4:
                        lo = (i - 1) * len(A1_dma) // 4
                        hi = i * len(A1_dma) // 4
                        mix += A1_dma[lo:hi]
                    if 5 <= i <= nbb - 2:
                        lo = (i - 5) * len(A1_vp) // (nbb - 6)
                        hi = (i - 4) * len(A1_vp) // (nbb - 6)
                        mix += A1_vp[lo:hi]
                    if i == nbb - 2:
                        mix += warm1
                elif b == 0 and 1 <= i <= nbb - 2:
                    lo = (i - 1) * len(A1) // (nbb - 2)
                    hi = i * len(A1) // (nbb - 2)
                    mix += A1[lo:hi]
                    if i == nbb - 2:
                        mix += warm1
                _interleave(primary, mix, lead=2)
                prev_av = av_tasks(b, h, qc, et)

            # ---- tail: final AVs, second half-AllToAll, output projection
            # ccdma slices whose data landed a block ago go first so their
            # transfer overlaps the final AV chain
            late_cc.sort(key=lambda x: x[0])
            for idx, task in late_cc:
                if idx == nblk:
                    task()
            for task in prev_av:
                task()
            for idx, task in late_cc:
                if idx > nblk:
                    task()
            for task in tail_tasks:
                task()
            t_a2a(1)()
            # vfull half B arrives row-chunked; all four DMAs are issued
            # up front (async on SP), each unblocking two projection tiles
            for rt in range(ROWS // P):
                t_vfull_rows(1, rt)()
            for rt in range(ROWS // P):
                oproj_rt(rt)()

        if loop_n > 1:
            t_wload(wv_sb, wv)()
            t_wload(wk_sb, wk)()
            t_wload(wq_sb, wq)()
            t_wload(wo_sb, wo)()
            if with_bias:
                t_bias_loads()()
            with tc.For_i(0, loop_n, 1):
                emit_body(load_weights=False)
        else:
            for rep in range(repeats):
                emit_body(load_weights=(rep == 0))

    nc.compile()
    return nc


def get_program(with_bias: bool, local_a2a: bool = False, xpose: str | None = None,
                repeats: int = 1, loop_n: int = 0, dve_cast: bool = False,
                qcw: int = QCW):
    key = (with_bias, local_a2a, xpose or XPOSE_MODE, repeats, loop_n, dve_cast, qcw)
    if key not in _CACHE:
        _CACHE[key] = _build_program(with_bias, local_a2a, xpose, repeats, loop_n,
                                     dve_cast, qcw)
    return _CACHE[key]


def make_in_maps(x, qkv_w, qkv_b, o_w, o_b):
    """Host-side sharding: slice per-head weight rows, transpose to the
    layouts the kernel consumes, cast weights to bf16."""
    bfnp = ml_dtypes.bfloat16
    x2 = np.ascontiguousarray(np.asarray(x, np.float32).reshape(BS, D))

    qkv_w = np.asarray(qkv_w, np.float32)
    o_w = np.asarray(o_w, np.float32)
    qkv_b = np.asarray(qkv_b, np.float32)
    o_b = np.asarray(o_b, np.float32)

    with_bias = bool(np.any(qkv_b) or np.any(o_b))

    woT = np.ascontiguousarray(
        o_w.T.reshape(NCORES, P, D).transpose(1, 0, 2).astype(bfnp)
    )
    ob_host = np.ascontiguousarray(o_b.reshape(1, D).astype(bfnp))

    in_maps = []
    for m in range(NCORES):
        heads = [m * HL + h for h in range(HL)]
        q_rows = np.concatenate([qkv_w[h * 3 * HD:h * 3 * HD + HD] for h in heads])
        k_rows = np.concatenate(
            [qkv_w[h * 3 * HD + HD:h * 3 * HD + 2 * HD] for h in heads]
        )
        v_rows = np.concatenate(
            [qkv_w[h * 3 * HD + 2 * HD:h * 3 * HD + 3 * HD] for h in heads]
        )

        def wt(rows):
            # [CH, D] -> [D, CH] -> [p, chunk, CH]
            return np.ascontiguousarray(
                rows.T.reshape(NDC, P, CH).transpose(1, 0, 2).astype(bfnp)
            )

        im = {
            "x": x2,
            "wq": wt(q_rows),
            "wk": wt(k_rows),
            "wv": wt(v_rows),
            "wo": woT,
        }
        if with_bias:
            bqv = np.concatenate(
                [qkv_b[h * 3 * HD:h * 3 * HD + HD] for h in heads]
            )
            bkv = np.concatenate(
                [qkv_b[h * 3 * HD + HD:h * 3 * HD + 2 * HD] for h in heads]
            )
            bvv = np.concatenate(
                [qkv_b[h * 3 * HD + 2 * HD:h * 3 * HD + 3 * HD] for h in heads]
            )
            im["bq"] = np.ascontiguousarray(bqv.reshape(1, CH).astype(bfnp))
            im["bk"] = np.ascontiguousarray(bkv.reshape(1, CH).astype(bfnp))
            im["bv"] = np.ascontiguousarray(bvv.reshape(1, CH).astype(bfnp))
            im["ob"] = ob_host
        in_maps.append(im)
    return in_maps, with_bias


def kernel(x, qkv_w, qkv_b, o_w, o_b):
    from concourse.bass_utils import run_bass_kernel_spmd

    in_maps, with_bias = make_in_maps(x, qkv_w, qkv_b, o_w, o_b)
    nc = get_program(with_bias)
    res = run_bass_kernel_spmd(nc, in_maps, list(range(NCORES)))
    out = np.concatenate([res.results[m]["y"] for m in range(NCORES)], axis=0)
    return np.ascontiguousarray(out.reshape(B, S, D))



# revision 50
# speedup vs baseline: 18.6253x; 3.3021x over previous
"""Multi-head attention (B=2, S=2048, D=1024, H=16) on 8 TRN2 NeuronCores.

Sharding: tensor-parallel over heads (2 heads/core).  Each core computes
the qkv projection for its heads (full sequence) and attention, then an
AllToAll redistributes attention outputs so each core holds *all* heads
for a 1/8 slice of the (batch*seq) rows and runs the output projection
locally.  No cross-core reduction needed.

Compute dtype: bf16 matmul operands, fp32 PSUM accumulation.  Softmax
denominators come for free from a ones-column appended to V (scores are
small here, so exp without max-subtraction is safe); normalization is a
per-partition scalar multiply fused into the PSUM eviction.

Engines execute their instruction streams in order, so the emission
order below is a hand-software-pipelined schedule: scores/exp of block
i+1 are interleaved with the attention-value matmuls of block i and
with the x-transpose/projection prep of the next batch.
"""

import sys

sys.path.insert(0, "/opt/trn_rl_repo")

import numpy as np
import ml_dtypes

B, S, D = 2, 2048, 1024
H, HD = 16, 64
NCORES = 8
BS = B * S                 # 4096 flattened rows
HL = H // NCORES           # 2 local heads
CH = HL * HD               # 128 local q/k/v channels
ROWS = BS // NCORES        # 512 output rows per core
P = 128
NDC = D // P               # 8 chunks of the contraction dim D
NST = S // P               # 16 seq tiles per batch
NKB = S // P               # 16 key blocks per batch
QCW = 1024                 # query-chunk width (one exp instruction per kb)
NQC = S // QCW             # query chunks per batch
HD1 = HD + 1               # value channels + ones column

_CACHE = {}

XPOSE_MODE = "pe"          # "pe" | "dma" (x transposed on the PE / via DMA)


def _interleave(primary, secondary, lead=0):
    """Emit primary tasks in order, spreading secondary tasks between them.
    The first `lead` primary tasks are emitted before any secondary."""
    ns = len(secondary)
    npr = max(len(primary) - lead, 1)
    si = 0
    for i, p in enumerate(primary):
        p()
        tgt = (i + 1 - lead) * ns // npr if i >= lead else 0
        while si < tgt:
            secondary[si]()
            si += 1
    while si < ns:
        secondary[si]()
        si += 1


def _build_program(with_bias: bool, local_a2a: bool = False, xpose: str | None = None,
                   repeats: int = 1, loop_n: int = 0, dve_cast: bool = False,
                   qcw: int = QCW):
    import concourse.bass as bass
    import concourse.mybir as mybir
    import concourse.tile as tile
    from concourse import bacc
    from concourse.masks import make_identity
    from contextlib import ExitStack

    xpose = xpose or XPOSE_MODE
    nqc = S // qcw
    nbb = HL * nqc          # blocks per batch
    dt = mybir.dt
    AF = mybir.ActivationFunctionType
    bf, f32 = dt.bfloat16, dt.float32

    nc = bacc.Bacc()

    x = nc.dram_tensor("x", [BS, D], f32, kind="ExternalInput")
    wq = nc.dram_tensor("wq", [P, NDC, CH], bf, kind="ExternalInput")
    wk = nc.dram_tensor("wk", [P, NDC, CH], bf, kind="ExternalInput")
    wv = nc.dram_tensor("wv", [P, NDC, CH], bf, kind="ExternalInput")
    wo = nc.dram_tensor("wo", [P, NCORES, D], bf, kind="ExternalInput")
    if with_bias:
        bq = nc.dram_tensor("bq", [1, CH], bf, kind="ExternalInput")
        bk = nc.dram_tensor("bk", [1, CH], bf, kind="ExternalInput")
        bv = nc.dram_tensor("bv", [1, CH], bf, kind="ExternalInput")
        ob = nc.dram_tensor("ob", [1, D], bf, kind="ExternalInput")
    y = nc.dram_tensor("y", [ROWS, D], f32, kind="ExternalOutput")

    # weight loads: HWDGE in pe mode; SWDGE in dma mode so the xbar
    # transposes don't interleave with copy-mode HWDGE transfers
    wload = (lambda **kw: nc.sync.dma_start(**kw)) if xpose == "pe" else (
        lambda **kw: nc.gpsimd.dma_start(**kw))

    with tile.TileContext(nc) as tc, ExitStack() as ctx:
        const = ctx.enter_context(tc.tile_pool(name="const", bufs=1))
        ident = const.tile([P, P], bf)
        make_identity(nc, ident[:])

        wq_sb = const.tile([P, NDC, CH], bf)
        wk_sb = const.tile([P, NDC, CH], bf)
        wv_sb = const.tile([P, NDC, CH], bf)
        wo_sb = const.tile([P, NCORES, D], bf)
        if with_bias:
            bq_sb = const.tile([1, CH], bf)
            bk_sb = const.tile([1, CH], bf)
            bv_sb = const.tile([1, CH], bf)
            ob_sb = const.tile([1, D], bf)
            ones_row = const.tile([1, 512], bf)

        big = ctx.enter_context(tc.tile_pool(name="big", bufs=1))
        xT = big.tile([P, NDC, BS], bf)                     # [d%128, d//128, row]
        qT = big.tile([P, BS], bf)                          # q channel-major
        kT = big.tile([P, BS], bf)                          # k channel-major
        v_aug = big.tile([P, B * NST, HL * HD1], bf)        # v row-major + ones
        valsT = big.tile([P, BS], bf)                       # attn out, ch-major

        xin = ctx.enter_context(tc.tile_pool(name="xin", bufs=4))
        expp = ctx.enter_context(tc.tile_pool(name="expp", bufs=(4 if qcw <= 512 else 2)))
        small = ctx.enter_context(tc.tile_pool(name="small", bufs=4))
        outp = ctx.enter_context(tc.tile_pool(name="outp", bufs=4))

        # PSUM budget: 8 banks total (bank-granular per buffer).
        #   qcw=512:  pt 2 + score 3 + proj 1 + av 2 = 8
        #   qcw=1024: pt 1 + score 2x2 + proj 1 + av 2 = 8
        nb_pt = 2 if qcw == 512 else 1
        nb_sc = 3 if qcw == 512 else 2
        pt = ctx.enter_context(tc.tile_pool(name="pt", bufs=nb_pt, space="PSUM"))
        pscore = ctx.enter_context(tc.tile_pool(name="pscore", bufs=nb_sc, space="PSUM"))
        pbig = ctx.enter_context(tc.tile_pool(name="pbig", bufs=1, space="PSUM"))
        pav = ctx.enter_context(tc.tile_pool(name="pav", bufs=2, space="PSUM"))

        dram = ctx.enter_context(tc.tile_pool(name="dram", bufs=1, space="DRAM"))
        # the AllToAll is split into two half-payload collectives (head 0 /
        # head 1 channel halves) so the first can run under live attention
        ccA_in = dram.tile([NCORES, HD, ROWS], bf)
        ccA_out = dram.tile([NCORES, HD, ROWS], bf)
        ccB_in = dram.tile([NCORES, HD, ROWS], bf)
        ccB_out = dram.tile([NCORES, HD, ROWS], bf)
        if xpose in ("dma", "hybrid"):
            xbf_dram = dram.tile([BS, D], bf)

        # ones columns for the softmax-denominator trick; value columns are
        # overwritten by the v-projection evictions
        for h in range(HL):
            nc.vector.memset(v_aug[:, :, h * HD1 + HD], 1.0)

        # ---------------- task builders ----------------

        def t_wload(wsb, wdram):
            return lambda: wload(out=wsb[:], in_=wdram[:])

        def t_bias_loads():
            def go():
                wload(out=bq_sb[:], in_=bq[:])
                wload(out=bk_sb[:], in_=bk[:])
                wload(out=bv_sb[:], in_=bv[:])
                wload(out=ob_sb[:], in_=ob[:])
                nc.vector.memset(ones_row[:], 1.0)
            return go

        xbufs = {}

        def t_xload(st):
            def go():
                x_bf = xin.tile([P, D], bf, tag="xbf", name="x_bf")
                xbufs[st] = x_bf
                nc.gpsimd.dma_start(out=x_bf[:], in_=x[st * P:(st + 1) * P, :])
            return go

        def t_xpose_pe(st):
            def go():
                x_bf = xbufs.pop(st)
                ptile = pt.tile([P, D], bf, tag="ptr", name="ptile")
                for c in range(NDC):
                    nc.tensor.transpose(
                        ptile[:, c * P:(c + 1) * P],
                        x_bf[:, c * P:(c + 1) * P], ident[:]
                    )
                # one wide PSUM->SBUF eviction per seq tile; dst free dims
                # (chunk, row-in-tile) match ptile's column order
                nc.vector.tensor_copy(
                    out=xT[:, :, st * P:(st + 1) * P], in_=ptile[:]
                )
            return go

        def t_xcast_dma(b, rc):
            def go():
                r0 = b * S + rc * 512
                nc.gpsimd.dma_start(
                    out=xbf_dram[r0:r0 + 512, :], in_=x[r0:r0 + 512, :]
                )
            return go

        def t_xpose_dma(b, rc, c):
            def go():
                r0 = b * S + rc * 512
                nc.sync.dma_start(
                    out=xT[:, c, r0:r0 + 512],
                    in_=xbf_dram[r0:r0 + 512, c * P:(c + 1) * P],
                    transpose=True,
                )
            return go

        def t_vproj4_parts(g):
            """v projection for seq tiles 4g..4g+3: four subtasks sharing
            one PSUM bank (disjoint 128-col accumulation groups); one
            strided eviction on the last.  NOTE: no other 'pk'-ring
            allocation may be emitted between the parts (single-buffer
            ring would head-of-line block the PE stream)."""
            state = {}

            def part(k):
                def go():
                    if k == 0:
                        state["pv"] = pbig.tile([P, 512], f32, tag="pk",
                                                name="pv4")
                    pv = state["pv"]
                    st = g * 4 + k
                    cs = pv[:, k * P:(k + 1) * P]
                    for c in range(NDC):
                        nc.tensor.matmul(
                            cs,
                            lhsT=xT[:, c, st * P:(st + 1) * P],
                            rhs=wv_sb[:, c, :],
                            start=(c == 0),
                            stop=(c == NDC - 1 and not with_bias),
                        )
                    if with_bias:
                        nc.tensor.matmul(
                            cs, lhsT=ones_row[:, 0:P], rhs=bv_sb[:],
                            start=False, stop=True,
                        )
                    if k == 3:
                        # out free dims (st, h, ch) / in (k, h, ch)
                        nc.vector.tensor_copy(
                            out=v_aug[:, g * 4:(g + 1) * 4, 0:HL * HD1]
                                .rearrange("p s (h c) -> p s h c",
                                           h=HL)[:, :, :, 0:HD],
                            in_=pv[:].rearrange("p (k h c) -> p k h c",
                                                k=4, h=HL),
                        )
                return go

            return [part(k) for k in range(4)]

        def t_kqproj(b, which, qc):
            def go():
                wsb, dst = (wk_sb, kT) if which == "k" else (wq_sb, qT)
                base = b * S + qc * 512
                pq = pbig.tile([P, 512], f32, tag="pk", name="pq")
                for c in range(NDC):
                    nc.tensor.matmul(
                        pq[:],
                        lhsT=wsb[:, c, :],
                        rhs=xT[:, c, base:base + 512],
                        start=(c == 0),
                        stop=(c == NDC - 1 and not with_bias),
                    )
                if with_bias:
                    nc.tensor.matmul(
                        pq[:],
                        lhsT=(bk_sb if which == "k" else bq_sb)[:],
                        rhs=ones_row[:],
                        start=False, stop=True,
                    )
                nc.vector.tensor_copy(out=dst[:, base:base + 512], in_=pq[:])
            return go

        def prep_A_tasks(b):
            """x load/cast/transpose + v projection, as (prefix, groups):
            groups[g] ends with the (split) v projection of seq tiles
            4g..4g+3 and all their transposes."""
            mode = xpose if xpose != "hybrid" else ("pe" if b == 0 else "dma")
            prefix, groups = [], []
            if mode == "pe":
                sts = [b * NST + t for t in range(NST)]
                prefix = [t_xload(sts[0]), t_xload(sts[1])]
                for g in range(4):
                    gt = []
                    for k in range(4):
                        i = g * 4 + k
                        if i + 2 < NST:
                            gt.append(t_xload(sts[i + 2]))
                        gt.append(t_xpose_pe(sts[i]))
                    gt += t_vproj4_parts(b * 4 + g)
                    groups.append(gt)
            else:
                for rc in range(4):
                    gt = [t_xcast_dma(b, rc)]
                    for c in range(NDC):
                        gt.append(t_xpose_dma(b, rc, c))
                    gt += t_vproj4_parts(b * 4 + rc)
                    groups.append(gt)
            return prefix, groups

        # attention blocks: per (b, h, qc) -> score tasks (one per kb) and
        # AV tasks (one per qt)
        def score_tasks(b, h, qc, et):
            hp = h * HD
            qbase = b * S + qc * qcw
            tasks = []

            def mk(kb):
                def go():
                    kbase = b * S + kb * P
                    ps = pscore.tile([P, qcw], f32, tag="ps", name="ps")
                    for qh in range(qcw // 512):
                        nc.tensor.matmul(
                            ps[:, qh * 512:(qh + 1) * 512],
                            lhsT=kT[hp:hp + HD, kbase:kbase + P],
                            rhs=qT[hp:hp + HD,
                                   qbase + qh * 512:qbase + (qh + 1) * 512],
                            start=True,
                            stop=True,
                        )
                    nc.scalar.activation(et[:, kb, :], ps[:], AF.Exp, scale=0.125)
                return go

            for kb in range(NKB):
                tasks.append(mk(kb))
            return tasks

        def av_tasks(b, h, qc, et):
            hp = h * HD
            qbase = b * S + qc * qcw
            tasks = []

            def mk(qt):
                def go():
                    pa = pav.tile([P, HD1], f32, tag="pa", name="pa")
                    for kb in range(NKB):
                        nc.tensor.matmul(
                            pa[:],
                            lhsT=et[:, kb, qt * P:(qt + 1) * P],
                            rhs=v_aug[:, b * NKB + kb, h * HD1:(h + 1) * HD1],
                            start=(kb == 0),
                            stop=(kb == NKB - 1),
                        )
                    rc_ = small.tile([P, 1], f32, tag="rc", name="rc")
                    nc.vector.reciprocal(rc_[:], pa[:, HD:HD1])
                    vn = small.tile([P, HD], bf, tag="vn", name="vn")
                    nc.vector.tensor_scalar_mul(vn[:], pa[:, 0:HD], rc_[:])
                    ptv = pt.tile([P, D], bf, tag="ptr", name="ptv")
                    nc.tensor.transpose(ptv[hp:hp + HD, 0:P], vn[:], ident[:])
                    col = qbase + qt * P
                    nc.vector.tensor_copy(
                        out=valsT[hp:hp + HD, col:col + P],
                        in_=ptv[hp:hp + HD, 0:P],
                    )
                return go

            for qt in range(qcw // P):
                tasks.append(mk(qt))
            return tasks

        def t_ccdma(half, j):
            ccin = ccA_in if half == 0 else ccB_in
            hp = half * HD
            return lambda: nc.sync.dma_start(
                out=ccin[j], in_=valsT[hp:hp + HD, j * ROWS:(j + 1) * ROWS]
            )

        def t_a2a(half):
            ccin, ccout = (ccA_in, ccA_out) if half == 0 else (ccB_in, ccB_out)

            def go():
                if local_a2a:
                    nc.sync.dma_start(out=ccout[:], in_=ccin[:])
                else:
                    nc.gpsimd.collective_compute(
                        "AllToAll",
                        mybir.AluOpType.bypass,
                        replica_groups=[list(range(NCORES))],
                        ins=[ccin[:]],
                        outs=[ccout[:]],
                    )
            return go

        # ---------------- emission (software pipeline) ----------------
        def emit_body(load_weights):
            A0pre, A0g = prep_A_tasks(0)
            # first x loads go out before the weight loads: they gate the
            # whole front, and run on the SWDGE queue anyway
            for task in A0pre:
                task()
            if load_weights:
                t_wload(wv_sb, wv)()
                t_wload(wk_sb, wk)()
                t_wload(wq_sb, wq)()
                if with_bias:
                    t_bias_loads()()
            A1pre, A1g = prep_A_tasks(1)
            A1 = A1pre + [t for g in A1g for t in g]
            # front: enough of batch 0 to start scoring, k/q chunks woven in
            for task in A0g[0]:         # seq tiles 0..3
                task()
            t_kqproj(0, "k", 0)()
            for task in A0g[1]:         # seq tiles 4..7
                task()
            t_kqproj(0, "q", 0)()
            if qcw > 512:
                t_kqproj(0, "q", 1)()
            if xpose != "pe":
                for g in (2, 3):
                    for task in A0g[g]:
                        task()

            # h-major block order per batch: the head-0 half of valsT is
            # complete after the last (b1,h0,*) block, letting the first
            # half-AllToAll run under the remaining head-1 attention.
            block_ids = [(b, h, qc) for b in range(B) for h in range(HL)
                         for qc in range(nqc)]
            nblk = len(block_ids)
            warm1 = [t_kqproj(1, "k", 0), t_kqproj(1, "q", 0)]
            if qcw > 512:
                warm1.append(t_kqproj(1, "q", 1))

            vfull = big.tile([P, NCORES, ROWS], bf, name="vfull")

            def t_vfull(half):
                ccout = ccA_out if half == 0 else ccB_out
                hp = half * HD
                return lambda: nc.sync.dma_start(
                    out=vfull[hp:hp + HD, :, :],
                    in_=ccout.rearrange("i p r -> p i r"),
                )

            def t_vfull_rows(half, rt):
                ccout = ccA_out if half == 0 else ccB_out
                hp = half * HD
                return lambda: nc.sync.dma_start(
                    out=vfull[hp:hp + HD, :, rt * P:(rt + 1) * P],
                    in_=ccout[:, :, rt * P:(rt + 1) * P].rearrange(
                        "i p r -> p i r"),
                )

            def _oproj_evict(rt, dh, pslice):
                osb = outp.tile([P, 512], f32, tag="osb", name="osb")
                nc.vector.tensor_copy(out=osb[:], in_=pslice)
                # y writes go out on the ACT HWDGE queue so they never
                # head-of-line block vfull loads on SP
                nc.scalar.dma_start(
                    out=y[rt * P:(rt + 1) * P, dh * 512:(dh + 1) * 512],
                    in_=osb[:],
                )

            def oproj_rt(rt):
                def go():
                    for dh in range(D // 512):
                        if qcw == 512:
                            # spread the 8 tiles over both [128,512] f32
                            # rings so the PE never waits on an eviction
                            pool, tg = ((pbig, "pk") if (rt * 2 + dh) % 4 == 3
                                        else (pscore, "ps"))
                            po = pool.tile([P, 512], f32, tag=tg, name="po")
                            pslice = po[:]
                        else:
                            if dh == 0:
                                po = pscore.tile([P, qcw], f32, tag="ps",
                                                 name="po")
                            pslice = po[:, dh * 512:(dh + 1) * 512]
                        for c in range(NCORES):
                            nc.tensor.matmul(
                                pslice,
                                lhsT=vfull[:, c, rt * P:(rt + 1) * P],
                                rhs=wo_sb[:, c, dh * 512:(dh + 1) * 512],
                                start=(c == 0),
                                stop=(c == NCORES - 1 and not with_bias),
                            )
                        if with_bias:
                            nc.tensor.matmul(
                                pslice, lhsT=ones_row[:, 0:P],
                                rhs=ob_sb[:, dh * 512:(dh + 1) * 512],
                                start=False, stop=True,
                            )
                        _oproj_evict(rt, dh, pslice)
                return go

            def oproj_half(rt, half, po):
                """Contraction split by head band: half 0 (channels 0:64 of
                every source core) accumulates while the B-half collective
                is still in flight; half 1 finishes and evicts."""
                hp = half * HD
                for dh in range(D // 512):
                    pslice = po[:, dh * 512:(dh + 1) * 512]
                    for c in range(NCORES):
                        nc.tensor.matmul(
                            pslice,
                            lhsT=vfull[hp:hp + HD, c, rt * P:(rt + 1) * P],
                            rhs=wo_sb[hp:hp + HD, c, dh * 512:(dh + 1) * 512],
                            start=(half == 0 and c == 0),
                            stop=(half == 1 and c == NCORES - 1
                                  and not with_bias),
                        )
                    if half == 1:
                        if with_bias:
                            nc.tensor.matmul(
                                pslice, lhsT=ones_row[:, 0:P],
                                rhs=ob_sb[:, dh * 512:(dh + 1) * 512],
                                start=False, stop=True,
                            )
                        _oproj_evict(rt, dh, pslice)

            # extra tasks joining the mix at a given global block index
            # (cc DMAs depend on AV tasks which lag their block by one)
            from collections import defaultdict
            extras = defaultdict(list)
            tail_tasks = []

            def sched(idx, task):
                if idx < nblk:
                    extras[idx].append(task)
                else:
                    tail_tasks.append(task)

            lastA = 0
            late_cc = []
            for b in range(B):
                for q in range(4):            # 512-row slot quarters
                    j = b * 4 + q
                    qc_of = q * 512 // qcw
                    blkA = b * nbb + qc_of
                    blkB = b * nbb + nqc + qc_of
                    sched(blkA + 2, t_ccdma(0, j))
                    if blkB + 2 < nblk:
                        sched(blkB + 2, t_ccdma(1, j))
                    else:
                        late_cc.append((blkB + 2, t_ccdma(1, j)))
                    lastA = max(lastA, blkA + 2)
            sched(lastA, t_a2a(0))
            sched(lastA, t_vfull(0))
            if load_weights:
                sched(nbb, t_wload(wo_sb, wo))
            # q-projection chunk c is emitted one block before the first
            # (b, h0, qc) block that reads it (chunk 0 — and 1 for wide
            # chunks — comes from the front / warm1 instead)
            for b in range(B):
                for c in range(1, 4):
                    first_qc = c * 512 // qcw
                    if first_qc == 0:
                        continue
                    if (b == 0 and first_qc == 1 and qcw > 512
                            and xpose == "pe"):
                        # block 0's mix interleaves from the start — these
                        # chunks contract seq tiles transposed late in block
                        # 0's primary, so they're woven there instead
                        continue
                    extras[b * nbb + first_qc - 1].insert(
                        0, t_kqproj(b, "q", c))

            prev_av = []
            for i, (b, h, qc) in enumerate(block_ids):
                et = expp.tile([P, NKB, qcw], bf, tag="exp", name="et")
                s = score_tasks(b, h, qc, et)
                if h == 0 and qc == 0:
                    kp = [t_kqproj(b, "k", c) for c in (1, 2, 3)]
                    if b == 0 and xpose == "pe":
                        # explicit weave: remaining A groups + k chunks after
                        # the A tiles they contract over (scores kb 4c..4c+3
                        # need k chunk c <- seq tiles 4c..4c+3); v-part runs
                        # stay contiguous (see t_vproj4_parts)
                        qp = ([t_kqproj(0, "q", 2), t_kqproj(0, "q", 3)]
                              if qcw > 512 else [])
                        g2, g3 = A0g[2], A0g[3]
                        primary = (s[0:2] + g2[:-4] + s[2:4] + kp[0:1]
                                   + g2[-4:] + s[4:6] + g3[:-4] + s[6:8]
                                   + kp[1:2] + g3[-4:] + qp + s[8:12]
                                   + kp[2:3] + s[12:16])
                    else:
                        primary = (s[0:4] + kp[0:1] + s[4:8] + kp[1:2]
                                   + s[8:12] + kp[2:3] + s[12:16])
                else:
                    primary = s
                mix = extras.get(i, [])[:]
                mix += prev_av
                if b == 0 and 1 <= i <= nbb - 2:
                    lo = (i - 1) * len(A1) // (nbb - 2)
                    hi = i * len(A1) // (nbb - 2)
                    mix += A1[lo:hi]
                    if i == nbb - 2:
                        mix += warm1
                _interleave(primary, mix, lead=2)
                prev_av = av_tasks(b, h, qc, et)

            # ---- tail: final AVs, second half-AllToAll, output projection
            # ccdma slices whose data landed a block ago go first so their
            # transfer overlaps the final AV chain
            late_cc.sort(key=lambda x: x[0])
            for idx, task in late_cc:
                if idx == nblk:
                    task()
            for task in prev_av:
                task()
            for idx, task in late_cc:
                if idx > nblk:
                    task()
            for task in tail_tasks:
                task()
            t_a2a(1)()
            # vfull half B arrives row-chunked; all four DMAs are issued
            # up front (async on SP), each unblocking two projection tiles
            for rt in range(ROWS // P):
                t_vfull_rows(1, rt)()
            if qcw > 512:
                # keep the PE busy through the collective: head-A halves of
                # the first two row tiles accumulate into held PSUM tiles,
                # then the B-halves land on top once their rows arrive
                pos = {}
                for rt in (0, 1):
                    pos[rt] = pscore.tile([P, qcw], f32, tag="ps", name="po")
                    oproj_half(rt, 0, pos[rt])
                for rt in (0, 1):
                    oproj_half(rt, 1, pos[rt])
                for rt in (2, 3):
                    oproj_rt(rt)()
            else:
                for rt in range(ROWS // P):
                    oproj_rt(rt)()

        if loop_n > 1:
            t_wload(wv_sb, wv)()
            t_wload(wk_sb, wk)()
            t_wload(wq_sb, wq)()
            t_wload(wo_sb, wo)()
            if with_bias:
                t_bias_loads()()
            with tc.For_i(0, loop_n, 1):
                emit_body(load_weights=False)
        else:
            for rep in range(repeats):
                emit_body(load_weights=(rep == 0))

    nc.compile()
    return nc


def get_program(with_bias: bool, local_a2a: bool = False, xpose: str | None = None,
                repeats: int = 1, loop_n: int = 0, dve_cast: bool = False,
                qcw: int = QCW):
    key = (with_bias, local_a2a, xpose or XPOSE_MODE, repeats, loop_n, dve_cast, qcw)
    if key not in _CACHE:
        _CACHE[key] = _build_program(with_bias, local_a2a, xpose, repeats, loop_n,
                                     dve_cast, qcw)
    return _CACHE[key]


def make_in_maps(x, qkv_w, qkv_b, o_w, o_b):
    """Host-side sharding: slice per-head weight rows, transpose to the
    layouts the kernel consumes, cast weights to bf16."""
    bfnp = ml_dtypes.bfloat16
    x2 = np.ascontiguousarray(np.asarray(x, np.float32).reshape(BS, D))

    qkv_w = np.asarray(qkv_w, np.float32)
    o_w = np.asarray(o_w, np.float32)
    qkv_b = np.asarray(qkv_b, np.float32)
    o_b = np.asarray(o_b, np.float32)

    with_bias = bool(np.any(qkv_b) or np.any(o_b))

    woT = np.ascontiguousarray(
        o_w.T.reshape(NCORES, P, D).transpose(1, 0, 2).astype(bfnp)
    )
    ob_host = np.ascontiguousarray(o_b.reshape(1, D).astype(bfnp))

    in_maps = []
    for m in range(NCORES):
        heads = [m * HL + h for h in range(HL)]
        q_rows = np.concatenate([qkv_w[h * 3 * HD:h * 3 * HD + HD] for h in heads])
        k_rows = np.concatenate(
            [qkv_w[h * 3 * HD + HD:h * 3 * HD + 2 * HD] for h in heads]
        )
        v_rows = np.concatenate(
            [qkv_w[h * 3 * HD + 2 * HD:h * 3 * HD + 3 * HD] for h in heads]
        )

        def wt(rows):
            # [CH, D] -> [D, CH] -> [p, chunk, CH]
            return np.ascontiguousarray(
                rows.T.reshape(NDC, P, CH).transpose(1, 0, 2).astype(bfnp)
            )

        im = {
            "x": x2,
            "wq": wt(q_rows),
            "wk": wt(k_rows),
            "wv": wt(v_rows),
            "wo": woT,
        }
        if with_bias:
            bqv = np.concatenate(
                [qkv_b[h * 3 * HD:h * 3 * HD + HD] for h in heads]
            )
            bkv = np.concatenate(
                [qkv_b[h * 3 * HD + HD:h * 3 * HD + 2 * HD] for h in heads]
            )
            bvv = np.concatenate(
                [qkv_b[h * 3 * HD + 2 * HD:h * 3 * HD + 3 * HD] for h in heads]
            )
            im["bq"] = np.ascontiguousarray(bqv.reshape(1, CH).astype(bfnp))
            im["bk"] = np.ascontiguousarray(bkv.reshape(1, CH).astype(bfnp))
            im["bv"] = np.ascontiguousarray(bvv.reshape(1, CH).astype(bfnp))
            im["ob"] = ob_host
        in_maps.append(im)
    return in_maps, with_bias


def kernel(x, qkv_w, qkv_b, o_w, o_b):
    from concourse.bass_utils import run_bass_kernel_spmd

    in_maps, with_bias = make_in_maps(x, qkv_w, qkv_b, o_w, o_b)
    nc = get_program(with_bias)
    res = run_bass_kernel_spmd(nc, in_maps, list(range(NCORES)))
    out = np.concatenate([res.results[m]["y"] for m in range(NCORES)], axis=0)
    return np.ascontiguousarray(out.reshape(B, S, D))



# revision 51
# speedup vs baseline: 24.5345x; 1.3173x over previous
"""Multi-head attention (B=2, S=2048, D=1024, H=16) on 8 TRN2 NeuronCores.

Sharding: tensor-parallel over heads (2 heads/core).  Each core computes
the qkv projection for its heads (full sequence) and attention, then an
AllToAll redistributes attention outputs so each core holds *all* heads
for a 1/8 slice of the (batch*seq) rows and runs the output projection
locally.  No cross-core reduction needed.

Compute dtype: bf16 matmul operands, fp32 PSUM accumulation.  Softmax
denominators come for free from a ones-column appended to V (scores are
small here, so exp without max-subtraction is safe); normalization is a
per-partition scalar multiply fused into the PSUM eviction.

Engines execute their instruction streams in order, so the emission
order below is a hand-software-pipelined schedule: scores/exp of block
i+1 are interleaved with the attention-value matmuls of block i and
with the x-transpose/projection prep of the next batch.  Exp runs in
1024-wide activation instructions (qcw=1024) to amortize the scalar
engine's per-instruction overhead — the exp stream is the steady-state
critical path of the attention phase.  The tail overlaps the second
half-AllToAll with the output projection: the head-0 contraction half
accumulates into held PSUM tiles while the head-1 payload is still in
flight, and vfull lands row-chunked so each chunk unblocks its row tile.
"""

import sys

sys.path.insert(0, "/opt/trn_rl_repo")

import numpy as np
import ml_dtypes

B, S, D = 2, 2048, 1024
H, HD = 16, 64
NCORES = 8
BS = B * S                 # 4096 flattened rows
HL = H // NCORES           # 2 local heads
CH = HL * HD               # 128 local q/k/v channels
ROWS = BS // NCORES        # 512 output rows per core
P = 128
NDC = D // P               # 8 chunks of the contraction dim D
NST = S // P               # 16 seq tiles per batch
NKB = S // P               # 16 key blocks per batch
QCW = 1024                 # query-chunk width (one exp instruction per kb)
NQC = S // QCW             # query chunks per batch
HD1 = HD + 1               # value channels + ones column

_CACHE = {}

XPOSE_MODE = "pe"          # "pe" | "dma" (x transposed on the PE / via DMA)


def _interleave(primary, secondary, lead=0):
    """Emit primary tasks in order, spreading secondary tasks between them.
    The first `lead` primary tasks are emitted before any secondary."""
    ns = len(secondary)
    npr = max(len(primary) - lead, 1)
    si = 0
    for i, p in enumerate(primary):
        p()
        tgt = (i + 1 - lead) * ns // npr if i >= lead else 0
        while si < tgt:
            secondary[si]()
            si += 1
    while si < ns:
        secondary[si]()
        si += 1


def _build_program(with_bias: bool, local_a2a: bool = False, xpose: str | None = None,
                   repeats: int = 1, loop_n: int = 0, dve_cast: bool = False,
                   qcw: int = QCW):
    import concourse.bass as bass
    import concourse.mybir as mybir
    import concourse.tile as tile
    from concourse import bacc
    from concourse.masks import make_identity
    from contextlib import ExitStack

    xpose = xpose or XPOSE_MODE
    nqc = S // qcw
    nbb = HL * nqc          # blocks per batch
    dt = mybir.dt
    AF = mybir.ActivationFunctionType
    bf, f32 = dt.bfloat16, dt.float32

    nc = bacc.Bacc()

    x = nc.dram_tensor("x", [BS, D], f32, kind="ExternalInput")
    wq = nc.dram_tensor("wq", [P, NDC, CH], bf, kind="ExternalInput")
    wk = nc.dram_tensor("wk", [P, NDC, CH], bf, kind="ExternalInput")
    wv = nc.dram_tensor("wv", [P, NDC, CH], bf, kind="ExternalInput")
    wo = nc.dram_tensor("wo", [P, NCORES, D], bf, kind="ExternalInput")
    if with_bias:
        bq = nc.dram_tensor("bq", [1, CH], bf, kind="ExternalInput")
        bk = nc.dram_tensor("bk", [1, CH], bf, kind="ExternalInput")
        bv = nc.dram_tensor("bv", [1, CH], bf, kind="ExternalInput")
        ob = nc.dram_tensor("ob", [1, D], bf, kind="ExternalInput")
    y = nc.dram_tensor("y", [ROWS, D], f32, kind="ExternalOutput")

    # weight loads: HWDGE in pe mode; SWDGE in dma mode so the xbar
    # transposes don't interleave with copy-mode HWDGE transfers
    wload = (lambda **kw: nc.sync.dma_start(**kw)) if xpose == "pe" else (
        lambda **kw: nc.gpsimd.dma_start(**kw))

    with tile.TileContext(nc) as tc, ExitStack() as ctx:
        const = ctx.enter_context(tc.tile_pool(name="const", bufs=1))
        ident = const.tile([P, P], bf)
        make_identity(nc, ident[:])

        wq_sb = const.tile([P, NDC, CH], bf)
        wk_sb = const.tile([P, NDC, CH], bf)
        wv_sb = const.tile([P, NDC, CH], bf)
        wo_sb = const.tile([P, NCORES, D], bf)
        if with_bias:
            bq_sb = const.tile([1, CH], bf)
            bk_sb = const.tile([1, CH], bf)
            bv_sb = const.tile([1, CH], bf)
            ob_sb = const.tile([1, D], bf)
            ones_row = const.tile([1, 512], bf)

        big = ctx.enter_context(tc.tile_pool(name="big", bufs=1))
        xT = big.tile([P, NDC, BS], bf)                     # [d%128, d//128, row]
        qT = big.tile([P, BS], bf)                          # q channel-major
        kT = big.tile([P, BS], bf)                          # k channel-major
        v_aug = big.tile([P, B * NST, HL * HD1], bf)        # v row-major + ones
        valsT = big.tile([P, BS], bf)                       # attn out, ch-major

        xin = ctx.enter_context(tc.tile_pool(name="xin", bufs=4))
        expp = ctx.enter_context(tc.tile_pool(name="expp", bufs=(4 if qcw <= 512 else 2)))
        small = ctx.enter_context(tc.tile_pool(name="small", bufs=4))
        outp = ctx.enter_context(tc.tile_pool(name="outp", bufs=4))

        # PSUM budget: 8 banks total (bank-granular per buffer).
        #   qcw=512:  pt 2 + score 3 + proj 1 + av 2 = 8
        #   qcw=1024: pt 1 + score 2x2 + proj 1 + av 2 = 8
        nb_pt = 2 if qcw == 512 else 1
        nb_sc = 3 if qcw == 512 else 2
        pt = ctx.enter_context(tc.tile_pool(name="pt", bufs=nb_pt, space="PSUM"))
        pscore = ctx.enter_context(tc.tile_pool(name="pscore", bufs=nb_sc, space="PSUM"))
        pbig = ctx.enter_context(tc.tile_pool(name="pbig", bufs=1, space="PSUM"))
        pav = ctx.enter_context(tc.tile_pool(name="pav", bufs=2, space="PSUM"))

        dram = ctx.enter_context(tc.tile_pool(name="dram", bufs=1, space="DRAM"))
        # the AllToAll is split into two half-payload collectives (head 0 /
        # head 1 channel halves) so the first can run under live attention
        ccA_in = dram.tile([NCORES, HD, ROWS], bf)
        ccA_out = dram.tile([NCORES, HD, ROWS], bf)
        ccB_in = dram.tile([NCORES, HD, ROWS], bf)
        ccB_out = dram.tile([NCORES, HD, ROWS], bf)
        if xpose in ("dma", "hybrid"):
            xbf_dram = dram.tile([BS, D], bf)

        # ones columns for the softmax-denominator trick; value columns are
        # overwritten by the v-projection evictions
        for h in range(HL):
            nc.vector.memset(v_aug[:, :, h * HD1 + HD], 1.0)

        # ---------------- task builders ----------------

        def t_wload(wsb, wdram):
            return lambda: wload(out=wsb[:], in_=wdram[:])

        def t_bias_loads():
            def go():
                wload(out=bq_sb[:], in_=bq[:])
                wload(out=bk_sb[:], in_=bk[:])
                wload(out=bv_sb[:], in_=bv[:])
                wload(out=ob_sb[:], in_=ob[:])
                nc.vector.memset(ones_row[:], 1.0)
            return go

        xbufs = {}

        def t_xload(st):
            def go():
                x_bf = xin.tile([P, D], bf, tag="xbf", name="x_bf")
                xbufs[st] = x_bf
                nc.gpsimd.dma_start(out=x_bf[:], in_=x[st * P:(st + 1) * P, :])
            return go

        def t_xpose_pe(st):
            def go():
                x_bf = xbufs.pop(st)
                ptile = pt.tile([P, D], bf, tag="ptr", name="ptile")
                for c in range(NDC):
                    nc.tensor.transpose(
                        ptile[:, c * P:(c + 1) * P],
                        x_bf[:, c * P:(c + 1) * P], ident[:]
                    )
                # one wide PSUM->SBUF eviction per seq tile; dst free dims
                # (chunk, row-in-tile) match ptile's column order
                nc.vector.tensor_copy(
                    out=xT[:, :, st * P:(st + 1) * P], in_=ptile[:]
                )
            return go

        def t_xcast_dma(b, rc):
            def go():
                r0 = b * S + rc * 512
                nc.gpsimd.dma_start(
                    out=xbf_dram[r0:r0 + 512, :], in_=x[r0:r0 + 512, :]
                )
            return go

        def t_xpose_dma(b, rc, c):
            def go():
                r0 = b * S + rc * 512
                nc.sync.dma_start(
                    out=xT[:, c, r0:r0 + 512],
                    in_=xbf_dram[r0:r0 + 512, c * P:(c + 1) * P],
                    transpose=True,
                )
            return go

        def t_vproj4_parts(g):
            """v projection for seq tiles 4g..4g+3: four subtasks sharing
            one PSUM bank (disjoint 128-col accumulation groups); one
            strided eviction on the last.  NOTE: no other 'pk'-ring
            allocation may be emitted between the parts (single-buffer
            ring would head-of-line block the PE stream)."""
            state = {}

            def part(k):
                def go():
                    if k == 0:
                        state["pv"] = pbig.tile([P, 512], f32, tag="pk",
                                                name="pv4")
                    pv = state["pv"]
                    st = g * 4 + k
                    cs = pv[:, k * P:(k + 1) * P]
                    for c in range(NDC):
                        nc.tensor.matmul(
                            cs,
                            lhsT=xT[:, c, st * P:(st + 1) * P],
                            rhs=wv_sb[:, c, :],
                            start=(c == 0),
                            stop=(c == NDC - 1 and not with_bias),
                        )
                    if with_bias:
                        nc.tensor.matmul(
                            cs, lhsT=ones_row[:, 0:P], rhs=bv_sb[:],
                            start=False, stop=True,
                        )
                    if k == 3:
                        # out free dims (st, h, ch) / in (k, h, ch)
                        nc.vector.tensor_copy(
                            out=v_aug[:, g * 4:(g + 1) * 4, 0:HL * HD1]
                                .rearrange("p s (h c) -> p s h c",
                                           h=HL)[:, :, :, 0:HD],
                            in_=pv[:].rearrange("p (k h c) -> p k h c",
                                                k=4, h=HL),
                        )
                return go

            return [part(k) for k in range(4)]

        def t_kqproj(b, which, qc):
            def go():
                wsb, dst = (wk_sb, kT) if which == "k" else (wq_sb, qT)
                base = b * S + qc * 512
                pq = pbig.tile([P, 512], f32, tag="pk", name="pq")
                for c in range(NDC):
                    nc.tensor.matmul(
                        pq[:],
                        lhsT=wsb[:, c, :],
                        rhs=xT[:, c, base:base + 512],
                        start=(c == 0),
                        stop=(c == NDC - 1 and not with_bias),
                    )
                if with_bias:
                    nc.tensor.matmul(
                        pq[:],
                        lhsT=(bk_sb if which == "k" else bq_sb)[:],
                        rhs=ones_row[:],
                        start=False, stop=True,
                    )
                nc.vector.tensor_copy(out=dst[:, base:base + 512], in_=pq[:])
            return go

        def prep_A_tasks(b):
            """x load/cast/transpose + v projection, as (prefix, groups):
            groups[g] ends with the (split) v projection of seq tiles
            4g..4g+3 and all their transposes."""
            mode = xpose if xpose != "hybrid" else ("pe" if b == 0 else "dma")
            prefix, groups = [], []
            if mode == "pe":
                sts = [b * NST + t for t in range(NST)]
                prefix = [t_xload(sts[0]), t_xload(sts[1])]
                for g in range(4):
                    gt = []
                    for k in range(4):
                        i = g * 4 + k
                        if i + 2 < NST:
                            gt.append(t_xload(sts[i + 2]))
                        gt.append(t_xpose_pe(sts[i]))
                    gt += t_vproj4_parts(b * 4 + g)
                    groups.append(gt)
            else:
                for rc in range(4):
                    gt = [t_xcast_dma(b, rc)]
                    for c in range(NDC):
                        gt.append(t_xpose_dma(b, rc, c))
                    gt += t_vproj4_parts(b * 4 + rc)
                    groups.append(gt)
            return prefix, groups

        # attention blocks: per (b, h, qc) -> score tasks (one per kb) and
        # AV tasks (one per qt)
        def score_tasks(b, h, qc, et):
            hp = h * HD
            qbase = b * S + qc * qcw
            tasks = []

            def mk(kb):
                def go():
                    kbase = b * S + kb * P
                    ps = pscore.tile([P, qcw], f32, tag="ps", name="ps")
                    for qh in range(qcw // 512):
                        nc.tensor.matmul(
                            ps[:, qh * 512:(qh + 1) * 512],
                            lhsT=kT[hp:hp + HD, kbase:kbase + P],
                            rhs=qT[hp:hp + HD,
                                   qbase + qh * 512:qbase + (qh + 1) * 512],
                            start=True,
                            stop=True,
                        )
                    nc.scalar.activation(et[:, kb, :], ps[:], AF.Exp, scale=0.125)
                return go

            for kb in range(NKB):
                tasks.append(mk(kb))
            return tasks

        def av_tasks(b, h, qc, et):
            hp = h * HD
            qbase = b * S + qc * qcw
            tasks = []

            def mk(qt):
                def go():
                    pa = pav.tile([P, HD1], f32, tag="pa", name="pa")
                    for kb in range(NKB):
                        nc.tensor.matmul(
                            pa[:],
                            lhsT=et[:, kb, qt * P:(qt + 1) * P],
                            rhs=v_aug[:, b * NKB + kb, h * HD1:(h + 1) * HD1],
                            start=(kb == 0),
                            stop=(kb == NKB - 1),
                        )
                    rc_ = small.tile([P, 1], f32, tag="rc", name="rc")
                    nc.vector.reciprocal(rc_[:], pa[:, HD:HD1])
                    vn = small.tile([P, HD], bf, tag="vn", name="vn")
                    nc.vector.tensor_scalar_mul(vn[:], pa[:, 0:HD], rc_[:])
                    ptv = pt.tile([P, D], bf, tag="ptr", name="ptv")
                    nc.tensor.transpose(ptv[hp:hp + HD, 0:P], vn[:], ident[:])
                    col = qbase + qt * P
                    nc.vector.tensor_copy(
                        out=valsT[hp:hp + HD, col:col + P],
                        in_=ptv[hp:hp + HD, 0:P],
                    )
                return go

            for qt in range(qcw // P):
                tasks.append(mk(qt))
            return tasks

        def t_ccdma(half, j):
            ccin = ccA_in if half == 0 else ccB_in
            hp = half * HD
            return lambda: nc.sync.dma_start(
                out=ccin[j], in_=valsT[hp:hp + HD, j * ROWS:(j + 1) * ROWS]
            )

        def t_a2a(half):
            ccin, ccout = (ccA_in, ccA_out) if half == 0 else (ccB_in, ccB_out)

            def go():
                if local_a2a:
                    nc.sync.dma_start(out=ccout[:], in_=ccin[:])
                else:
                    nc.gpsimd.collective_compute(
                        "AllToAll",
                        mybir.AluOpType.bypass,
                        replica_groups=[list(range(NCORES))],
                        ins=[ccin[:]],
                        outs=[ccout[:]],
                    )
            return go

        # ---------------- emission (software pipeline) ----------------
        def emit_body(load_weights):
            A0pre, A0g = prep_A_tasks(0)
            # first x loads go out before the weight loads: they gate the
            # whole front, and run on the SWDGE queue anyway
            for task in A0pre:
                task()
            if load_weights:
                t_wload(wv_sb, wv)()
                t_wload(wk_sb, wk)()
                t_wload(wq_sb, wq)()
                if with_bias:
                    t_bias_loads()()
            A1pre, A1g = prep_A_tasks(1)
            A1 = A1pre + [t for g in A1g for t in g]
            # front: enough of batch 0 to start scoring, k/q chunks woven in
            for task in A0g[0]:         # seq tiles 0..3
                task()
            t_kqproj(0, "k", 0)()
            for task in A0g[1]:         # seq tiles 4..7
                task()
            t_kqproj(0, "q", 0)()
            if qcw > 512:
                t_kqproj(0, "q", 1)()
            if xpose != "pe":
                for g in (2, 3):
                    for task in A0g[g]:
                        task()

            # h-major block order per batch: the head-0 half of valsT is
            # complete after the last (b1,h0,*) block, letting the first
            # half-AllToAll run under the remaining head-1 attention.
            block_ids = [(b, h, qc) for b in range(B) for h in range(HL)
                         for qc in range(nqc)]
            nblk = len(block_ids)
            warm1 = [t_kqproj(1, "k", 0), t_kqproj(1, "q", 0)]
            if qcw > 512:
                warm1.append(t_kqproj(1, "q", 1))

            vfull = big.tile([P, NCORES, ROWS], bf, name="vfull")

            def t_vfull(half):
                ccout = ccA_out if half == 0 else ccB_out
                hp = half * HD
                return lambda: nc.sync.dma_start(
                    out=vfull[hp:hp + HD, :, :],
                    in_=ccout.rearrange("i p r -> p i r"),
                )

            def t_vfull_rows(half, rt):
                ccout = ccA_out if half == 0 else ccB_out
                hp = half * HD
                return lambda: nc.sync.dma_start(
                    out=vfull[hp:hp + HD, :, rt * P:(rt + 1) * P],
                    in_=ccout[:, :, rt * P:(rt + 1) * P].rearrange(
                        "i p r -> p i r"),
                )

            def _oproj_evict(rt, dh, pslice):
                osb = outp.tile([P, 512], f32, tag="osb", name="osb")
                nc.vector.tensor_copy(out=osb[:], in_=pslice)
                # y writes go out on the ACT HWDGE queue so they never
                # head-of-line block vfull loads on SP
                nc.scalar.dma_start(
                    out=y[rt * P:(rt + 1) * P, dh * 512:(dh + 1) * 512],
                    in_=osb[:],
                )

            def oproj_rt(rt):
                def go():
                    for dh in range(D // 512):
                        if qcw == 512:
                            # spread the 8 tiles over both [128,512] f32
                            # rings so the PE never waits on an eviction
                            pool, tg = ((pbig, "pk") if (rt * 2 + dh) % 4 == 3
                                        else (pscore, "ps"))
                            po = pool.tile([P, 512], f32, tag=tg, name="po")
                            pslice = po[:]
                        else:
                            if dh == 0:
                                po = pscore.tile([P, qcw], f32, tag="ps",
                                                 name="po")
                            pslice = po[:, dh * 512:(dh + 1) * 512]
                        for c in range(NCORES):
                            nc.tensor.matmul(
                                pslice,
                                lhsT=vfull[:, c, rt * P:(rt + 1) * P],
                                rhs=wo_sb[:, c, dh * 512:(dh + 1) * 512],
                                start=(c == 0),
                                stop=(c == NCORES - 1 and not with_bias),
                            )
                        if with_bias:
                            nc.tensor.matmul(
                                pslice, lhsT=ones_row[:, 0:P],
                                rhs=ob_sb[:, dh * 512:(dh + 1) * 512],
                                start=False, stop=True,
                            )
                        _oproj_evict(rt, dh, pslice)
                return go

            def oproj_half(rt, half, po):
                """Contraction split by head band: half 0 (channels 0:64 of
                every source core) accumulates while the B-half collective
                is still in flight; half 1 finishes and evicts."""
                hp = half * HD
                for dh in range(D // 512):
                    pslice = po[:, dh * 512:(dh + 1) * 512]
                    for c in range(NCORES):
                        nc.tensor.matmul(
                            pslice,
                            lhsT=vfull[hp:hp + HD, c, rt * P:(rt + 1) * P],
                            rhs=wo_sb[hp:hp + HD, c, dh * 512:(dh + 1) * 512],
                            start=(half == 0 and c == 0),
                            stop=(half == 1 and c == NCORES - 1
                                  and not with_bias),
                        )
                    if half == 1:
                        if with_bias:
                            nc.tensor.matmul(
                                pslice, lhsT=ones_row[:, 0:P],
                                rhs=ob_sb[:, dh * 512:(dh + 1) * 512],
                                start=False, stop=True,
                            )
                        _oproj_evict(rt, dh, pslice)

            # extra tasks joining the mix at a given global block index
            # (cc DMAs depend on AV tasks which lag their block by one)
            from collections import defaultdict
            extras = defaultdict(list)
            tail_tasks = []

            def sched(idx, task):
                if idx < nblk:
                    extras[idx].append(task)
                else:
                    tail_tasks.append(task)

            lastA = 0
            late_cc = []
            for b in range(B):
                for q in range(4):            # 512-row slot quarters
                    j = b * 4 + q
                    qc_of = q * 512 // qcw
                    blkA = b * nbb + qc_of
                    blkB = b * nbb + nqc + qc_of
                    sched(blkA + 2, t_ccdma(0, j))
                    if blkB + 2 < nblk:
                        sched(blkB + 2, t_ccdma(1, j))
                    else:
                        late_cc.append((blkB + 2, t_ccdma(1, j)))
                    lastA = max(lastA, blkA + 2)
            sched(lastA, t_a2a(0))
            sched(lastA, t_vfull(0))
            if load_weights:
                sched(nbb, t_wload(wo_sb, wo))
            # q-projection chunk c is emitted one block before the first
            # (b, h0, qc) block that reads it (chunk 0 — and 1 for wide
            # chunks — comes from the front / warm1 instead)
            for b in range(B):
                for c in range(1, 4):
                    first_qc = c * 512 // qcw
                    if first_qc == 0:
                        continue
                    if (b == 0 and first_qc == 1 and qcw > 512
                            and xpose == "pe"):
                        # block 0's mix interleaves from the start — these
                        # chunks contract seq tiles transposed late in block
                        # 0's primary, so they're woven there instead
                        continue
                    extras[b * nbb + first_qc - 1].insert(
                        0, t_kqproj(b, "q", c))

            prev_av = []
            for i, (b, h, qc) in enumerate(block_ids):
                et = expp.tile([P, NKB, qcw], bf, tag="exp", name="et")
                s = score_tasks(b, h, qc, et)
                if h == 0 and qc == 0:
                    kp = [t_kqproj(b, "k", c) for c in (1, 2, 3)]
                    if b == 0 and xpose == "pe":
                        # explicit weave: remaining A groups + k chunks after
                        # the A tiles they contract over (scores kb 4c..4c+3
                        # need k chunk c <- seq tiles 4c..4c+3); v-part runs
                        # stay contiguous (see t_vproj4_parts)
                        qp = ([t_kqproj(0, "q", 2), t_kqproj(0, "q", 3)]
                              if qcw > 512 else [])
                        g2, g3 = A0g[2], A0g[3]
                        primary = (s[0:2] + g2[:-4] + s[2:4] + kp[0:1]
                                   + g2[-4:] + s[4:6] + g3[:-4] + s[6:8]
                                   + kp[1:2] + g3[-4:] + qp + s[8:12]
                                   + kp[2:3] + s[12:16])
                    else:
                        primary = (s[0:4] + kp[0:1] + s[4:8] + kp[1:2]
                                   + s[8:12] + kp[2:3] + s[12:16])
                else:
                    primary = s
                mix = extras.get(i, [])[:]
                mix += prev_av
                if b == 0 and 1 <= i <= nbb - 2:
                    lo = (i - 1) * len(A1) // (nbb - 2)
                    hi = i * len(A1) // (nbb - 2)
                    mix += A1[lo:hi]
                    if i == nbb - 2:
                        mix += warm1
                _interleave(primary, mix, lead=2)
                prev_av = av_tasks(b, h, qc, et)

            # ---- tail: final AVs, second half-AllToAll, output projection
            # ccdma slices whose data landed a block ago go first so their
            # transfer overlaps the final AV chain
            late_cc.sort(key=lambda x: x[0])
            for idx, task in late_cc:
                if idx == nblk:
                    task()
            for task in prev_av:
                task()
            for idx, task in late_cc:
                if idx > nblk:
                    task()
            for task in tail_tasks:
                task()
            t_a2a(1)()
            # vfull half B arrives row-chunked; all four DMAs are issued
            # up front (async on SP), each unblocking two projection tiles
            for rt in range(ROWS // P):
                t_vfull_rows(1, rt)()
            if qcw > 512:
                # keep the PE busy through the collective: head-A halves of
                # the first two row tiles accumulate into held PSUM tiles,
                # then the B-halves land on top once their rows arrive
                pos = {}
                for rt in (0, 1):
                    pos[rt] = pscore.tile([P, qcw], f32, tag="ps", name="po")
                    oproj_half(rt, 0, pos[rt])
                for rt in (0, 1):
                    oproj_half(rt, 1, pos[rt])
                for rt in (2, 3):
                    oproj_rt(rt)()
            else:
                for rt in range(ROWS // P):
                    oproj_rt(rt)()

        if loop_n > 1:
            t_wload(wv_sb, wv)()
            t_wload(wk_sb, wk)()
            t_wload(wq_sb, wq)()
            t_wload(wo_sb, wo)()
            if with_bias:
                t_bias_loads()()
            with tc.For_i(0, loop_n, 1):
                emit_body(load_weights=False)
        else:
            for rep in range(repeats):
                emit_body(load_weights=(rep == 0))

    nc.compile()
    return nc


def get_program(with_bias: bool, local_a2a: bool = False, xpose: str | None = None,
                repeats: int = 1, loop_n: int = 0, dve_cast: bool = False,
                qcw: int = QCW):
    key = (with_bias, local_a2a, xpose or XPOSE_MODE, repeats, loop_n, dve_cast, qcw)
    if key not in _CACHE:
        _CACHE[key] = _build_program(with_bias, local_a2a, xpose, repeats, loop_n,
                                     dve_cast, qcw)
    return _CACHE[key]


def make_in_maps(x, qkv_w, qkv_b, o_w, o_b):
    """Host-side sharding: slice per-head weight rows, transpose to the
    layouts the kernel consumes, cast weights to bf16."""
    bfnp = ml_dtypes.bfloat16
    x2 = np.ascontiguousarray(np.asarray(x, np.float32).reshape(BS, D))

    qkv_w = np.asarray(qkv_w, np.float32)
    o_w = np.asarray(o_w, np.float32)
    qkv_b = np.asarray(qkv_b, np.float32)
    o_b = np.asarray(o_b, np.float32)

    with_bias = bool(np.any(qkv_b) or np.any(o_b))

    woT = np.ascontiguousarray(
        o_w.T.reshape(NCORES, P, D).transpose(1, 0, 2).astype(bfnp)
    )
    ob_host = np.ascontiguousarray(o_b.reshape(1, D).astype(bfnp))

    in_maps = []
    for m in range(NCORES):
        heads = [m * HL + h for h in range(HL)]
        q_rows = np.concatenate([qkv_w[h * 3 * HD:h * 3 * HD + HD] for h in heads])
        k_rows = np.concatenate(
            [qkv_w[h * 3 * HD + HD:h * 3 * HD + 2 * HD] for h in heads]
        )
        v_rows = np.concatenate(
            [qkv_w[h * 3 * HD + 2 * HD:h * 3 * HD + 3 * HD] for h in heads]
        )

        def wt(rows):
            # [CH, D] -> [D, CH] -> [p, chunk, CH]
            return np.ascontiguousarray(
                rows.T.reshape(NDC, P, CH).transpose(1, 0, 2).astype(bfnp)
            )

        im = {
            "x": x2,
            "wq": wt(q_rows),
            "wk": wt(k_rows),
            "wv": wt(v_rows),
            "wo": woT,
        }
        if with_bias:
            bqv = np.concatenate(
                [qkv_b[h * 3 * HD:h * 3 * HD + HD] for h in heads]
            )
            bkv = np.concatenate(
                [qkv_b[h * 3 * HD + HD:h * 3 * HD + 2 * HD] for h in heads]
            )
            bvv = np.concatenate(
                [qkv_b[h * 3 * HD + 2 * HD:h * 3 * HD + 3 * HD] for h in heads]
            )
            im["bq"] = np.ascontiguousarray(bqv.reshape(1, CH).astype(bfnp))
            im["bk"] = np.ascontiguousarray(bkv.reshape(1, CH).astype(bfnp))
            im["bv"] = np.ascontiguousarray(bvv.reshape(1, CH).astype(bfnp))
            im["ob"] = ob_host
        in_maps.append(im)
    return in_maps, with_bias


def kernel(x, qkv_w, qkv_b, o_w, o_b):
    from concourse.bass_utils import run_bass_kernel_spmd

    in_maps, with_bias = make_in_maps(x, qkv_w, qkv_b, o_w, o_b)
    nc = get_program(with_bias)
    res = run_bass_kernel_spmd(nc, in_maps, list(range(NCORES)))
    out = np.concatenate([res.results[m]["y"] for m in range(NCORES)], axis=0)
    return np.ascontiguousarray(out.reshape(B, S, D))

